# revision 9
# baseline (speedup 1.0000x reference)
"""MoE (top-2 of 8 experts + 1 shared expert, SwiGLU FFN) on 8 TRN2 NeuronCores.

Strategy (expert-parallel + load-balanced overflow, per the sharding hint):
  - Host computes the (tiny) gate: softmax top-2 over E=8 for T=8192 tokens,
    and from it the dispatch. >99.9% of FLOPs (the FFNs) run on device.
  - Load balancing: expert loads vary (~1932..2182 for T=8192); SPMD padding
    to the max would cost ~5%. Instead each core runs its own expert's first
    R tokens (R ~ 1980, two half-passes) plus a small "borrowed" group
    (cap cB ~ 104) of overflow tokens from some hot expert, with that
    expert's weights supplied per-core. (R, cB) solve
    min R+cB s.t. sum_e ceil((c_e-R)+ / cB) <= 8, bringing per-core routed
    work from max_e c_e (~2184) down to ~mean+2% (~2084).
  - The borrowed group rides INSIDE half-pass 1 as a second token group:
    its weight stream (w1b/w2b, 24MB) amortizes over the whole half-pass.
    A standalone borrowed pass would need >500GB/s of weight bandwidth.
  - Each half-pass scales rows by the gate weight and scatters rows into an
    AllToAll dispatch buffer laid out by destination core; each half's
    AllToAll fires when ready, overlapping remaining compute. Each core also
    runs the shared expert on its own T/8 token slice.
  - Combine on device: out[t] = shared(t) + contrib0(t) + contrib1(t), where
    contribs are indirect-gathered from the a2a output segments (host
    precomputes absolute rows; placement is fully host-controlled).

Compute dtype is fp16 (PSUM accumulation fp32). The PE clock is power-capped
at ~1.95 GHz sustained (GPIO throttle k=13/16), so the kernel is tuned to
keep the MM stream dense: all DMAs are large partition-major transfers
(weights are host-tiled so each f-chunk / w2-slab is a single DMA), biases
are folded into the PSUM->SBUF drain, the per-row gate scale runs on the
scalar engine, and the finalize/combine work is interleaved per-tile with
the last stage-2 slab so nothing serializes after the final matmul. At
phase start, x chunks >=1 and the slab-0 w2 load are emitted behind the
first weight tiles so the first matmuls aren't queued behind bulk DMAs.
"""
import contextlib

import numpy as np

import concourse.bass as bass
import concourse.tile as tile
from concourse import bacc, mybir
from concourse.bass_utils import run_bass_kernel_spmd

# problem shape (hardcoded per contract)
T = 8192
D = 1024
F = 4096
E = 8
TOPK = 2
NCORES = 8
TO = T // NCORES          # tokens owned per core

F32 = mybir.dt.float32
F16 = mybir.dt.float16
I32 = mybir.dt.int32

MF = 2 * F // 128 // 2    # 32 f-chunks (a-half; b-half is mp+MF)
N_SLAB = 4
PER_SLAB = MF // N_SLAB   # 8 f-chunks per slab
KD = D // 128             # 8 contraction chunks for stage 1

_nc_cache: dict[tuple, object] = {}


def _chunk_slices(c_len):
    """Moving-dim chunks of <=512, each >=256 so LDWEIGHTS stays hidden.

    Largest chunk first: it gates the first matmul of a pass, and the
    extra matmul runway it provides covers the smaller chunks' transfers.
    """
    out = []
    rem = c_len
    while rem > 0:
        if rem <= 512:
            w = rem
        elif rem >= 768:
            w = 512
        else:
            w = rem - 256
        out.append(w)
        rem -= w
    out.sort(reverse=True)
    widths = []
    pos = 0
    for w in out:
        widths.append((pos, w))
        pos += w
    return widths


def _n_tiles(c_len):
    return -(-c_len // 128)


class _Group:
    """One token group within an FFN pass: its own x, weights, biases,
    output tiles and finalize callback."""

    def __init__(self, x_src, c_len, w1d, w2d, b1t, b2t, y_tiles, on_tile_d,
                 w1_eng, w2_engs):
        self.x_src = x_src
        self.c_len = c_len
        self.w1d = w1d
        self.w2d = w2d
        self.b1t = b1t
        self.b2t = b2t
        self.y_tiles = y_tiles
        self.on_tile_d = on_tile_d
        self.w1_eng = w1_eng
        self.w2_engs = w2_engs
        self.chunks = _chunk_slices(c_len)
        self.xk = []
        self.g_tiles = []
        self.w2t = None


def _ffn_phase(nc, sbuf, psA, psB, groups, x_eng, pre_loads=None):
    """Emit one SwiGLU FFN pass over one or more token groups.

    Per group: x_src is per-chunk DRAM APs [128, KD, 512]; w1d is DRAM
    [MF, 128, 2, KD, 128] fp16 (host-tiled; [...,0,...]=a, 1=b); w2d is
    DRAM [N_SLAB, 128, PER_SLAB, D] fp16; b1t SBUF [128, 2*MF] f32 (col m =
    bias for a-chunk m / b-chunk m-MF); b2t SBUF [128, D] f32 added once
    into y at slab 0; y_tiles receive FFN output + bias2;
    on_tile_d(q, t, rows, d, ds, last) runs after each stage-2 add.

    Every x chunk / k-half and each w2 half lives in its own tile so the
    DMAs carry no false whole-tile dependencies and transfer concurrently.
    Only the first chunk's x is loaded ahead of the first weight tile; the
    rest are deferred so the first matmul isn't queued behind bulk DMAs.
    """
    KH = KD // 2
    ei = 0
    deferred_x = []
    for gi, g in enumerate(groups):
        g.xk = []
        for ci, (cs, cw) in enumerate(g.chunks):
            xa = sbuf.tile([128, KH, 512], F16, tag=f"x{gi}c{ci}a",
                           name=f"x{gi}c{ci}a", bufs=2 if gi == 0 else 1)
            xb = sbuf.tile([128, KH, 512], F16, tag=f"x{gi}c{ci}b",
                           name=f"x{gi}c{ci}b", bufs=2 if gi == 0 else 1)
            e0, e1 = x_eng[ei % len(x_eng)]
            ei += 1
            if gi == 0 and ci == 0:
                e0.dma_start(out=xa[:], in_=g.x_src[ci][:, 0:KH])
                e1.dma_start(out=xb[:], in_=g.x_src[ci][:, KH:KD])
            else:
                deferred_x.append((e0, xa, e1, xb, g.x_src[ci]))
            g.xk.append((xa, xb))
    if pre_loads is not None:
        pre_loads()

    for q in range(N_SLAB):
        for g in groups:
            g.g_tiles = []
        for fi in range(PER_SLAB):
            mp = q * PER_SLAB + fi
            for gi, g in enumerate(groups):
                w1t = sbuf.tile([128, 2, KD, 128], F16, tag=f"w1t{gi}",
                                name=f"w1t{gi}", bufs=6 if gi == 0 else 2)
                g.w1_eng.dma_start(out=w1t[:], in_=g.w1d[mp])
                if q == 0 and fi == 0 and gi == 0:
                    # x chunks beyond the first ride behind the first
                    # weight tile in queue order
                    for (e0, xa, e1, xb, src) in deferred_x:
                        e0.dma_start(out=xa[:], in_=src[:, 0:KH])
                        e1.dma_start(out=xb[:], in_=src[:, KH:KD])
                g_t = sbuf.tile([128, g.c_len], F16, tag=f"g{gi}_{fi}",
                                name=f"g{gi}_{fi}", bufs=1)
                for ci, (cs, cw) in enumerate(g.chunks):
                    ps_a = psA.tile([128, 512], F32, space="PSUM", tag="ps_a",
                                    name="ps_a", bufs=3)
                    ps_b = psA.tile([128, 512], F32, space="PSUM", tag="ps_b",
                                    name="ps_b", bufs=3)
                    for k in range(KD):
                        rhs = g.xk[ci][k // KH][:, k % KH, :cw]
                        nc.tensor.matmul(out=ps_a[:, :cw],
                                         lhsT=w1t[:, 0, k, :], rhs=rhs,
                                         start=(k == 0), stop=(k == KD - 1))
                    for k in range(KD):
                        rhs = g.xk[ci][k // KH][:, k % KH, :cw]
                        nc.tensor.matmul(out=ps_b[:, :cw],
                                         lhsT=w1t[:, 1, k, :], rhs=rhs,
                                         start=(k == 0), stop=(k == KD - 1))
                    t_a = sbuf.tile([128, 512], F16, tag="t_a", name="t_a",
                                    bufs=4)
                    t_b = sbuf.tile([128, 512], F16, tag="t_b", name="t_b",
                                    bufs=4)
                    nc.scalar.activation(t_a[:, :cw], ps_a[:, :cw],
                                         mybir.ActivationFunctionType.Silu,
                                         bias=g.b1t[:, mp:mp + 1])
                    nc.scalar.activation(
                        t_b[:, :cw], ps_b[:, :cw],
                        mybir.ActivationFunctionType.Identity,
                        bias=g.b1t[:, mp + MF:mp + MF + 1])
                    nc.vector.tensor_mul(g_t[:, cs:cs + cw], t_a[:, :cw],
                                         t_b[:, :cw])
                g.g_tiles.append(g_t)
            # stage-2 weights for this slab (streamed during stage-1).
            # Slab 0's load is emitted mid-pass so it doesn't sit in front
            # of the first weight tiles in the queue at phase start.
            if fi == (3 if q == 0 else 0):
                for gi, g in enumerate(groups):
                    w2ta = sbuf.tile([128, PER_SLAB // 2, D], F16,
                                     tag=f"w2{gi}a", name=f"w2{gi}a", bufs=1)
                    w2tb = sbuf.tile([128, PER_SLAB // 2, D], F16,
                                     tag=f"w2{gi}b", name=f"w2{gi}b", bufs=1)
                    g.w2_engs[0].dma_start(out=w2ta[:],
                                           in_=g.w2d[q][:, 0:PER_SLAB // 2])
                    g.w2_engs[1].dma_start(
                        out=w2tb[:], in_=g.w2d[q][:, PER_SLAB // 2:PER_SLAB])
                    g.w2t = (w2ta, w2tb)
        # stage-2 partial: y (+)= g_slab.T @ w2_slab
        for g in groups:
            n_t = _n_tiles(g.c_len)
            w2ta, w2tb = g.w2t
            for t in range(n_t):
                rows = min(128, g.c_len - t * 128)
                ts = slice(t * 128, t * 128 + rows)
                for d in range(D // 512):
                    ds = slice(d * 512, (d + 1) * 512)
                    ps_y = psB.tile([128, 512], F32, space="PSUM",
                                    tag="ps_y", name="ps_y", bufs=2)
                    for fi in range(PER_SLAB):
                        w2s = w2ta if fi < PER_SLAB // 2 else w2tb
                        nc.tensor.matmul(
                            out=ps_y[:rows, :], lhsT=g.g_tiles[fi][:, ts],
                            rhs=w2s[:, fi % (PER_SLAB // 2), ds],
                            start=(fi == 0), stop=(fi == PER_SLAB - 1))
                    yt = g.y_tiles[t]
                    if q == 0:
                        nc.vector.tensor_add(yt[:rows, ds], ps_y[:rows, :],
                                             g.b2t[:rows, ds])
                    else:
                        nc.vector.tensor_add(yt[:rows, ds], yt[:rows, ds],
                                             ps_y[:rows, :])
                    g.on_tile_d(q, t, rows, d, ds, d == D // 512 - 1)


def _build(c_r, c_b, p0):
    """SPMD program: half-pass 0 (own expert, c_r/2 tokens), half-pass 1
    (own c_r/2 + borrowed c_b as a second group), shared pass. Each half
    scatters into its own AllToAll buffer; the shared pass gathers +
    combines + stores."""
    key = (c_r, c_b, p0)
    if key in _nc_cache:
        return _nc_cache[key]

    nc = bacc.Bacc("TRN2", target_bir_lowering=False, debug=False,
                   num_devices=NCORES)

    def din(name, shape, dt):
        return nc.dram_tensor(name, shape, dt, kind="ExternalInput").ap()

    hR = c_r // 2
    n_thR = _n_tiles(hR)                               # y tiles per R half
    n_thB = _n_tiles(c_b)
    n_to = TO // 128
    G = 2 * n_thR + n_thB                              # cw/scat grid columns
    rows_h = NCORES * p0                               # rows per half buffer

    n_chR = len(_chunk_slices(hR))
    n_chB = len(_chunk_slices(c_b))
    n_cs = len(_chunk_slices(TO))
    # gathered/owned tokens^T, one contiguous 512-wide block per chunk
    xg0 = din("xg0", [n_chR, 128, KD, 512], F16)
    xg1 = din("xg1", [n_chR, 128, KD, 512], F16)
    xgb = din("xgb", [n_chB, 128, KD, 512], F16)
    xs = din("xs", [n_cs, 128, KD, 512], F16)
    w1o = din("w1o", [MF, 128, 2, KD, 128], F16)
    w2o = din("w2o", [N_SLAB, 128, PER_SLAB, D], F16)
    w1b = din("w1b", [MF, 128, 2, KD, 128], F16)
    w2b = din("w2b", [N_SLAB, 128, PER_SLAB, D], F16)
    sw1 = din("sw1", [MF, 128, 2, KD, 128], F16)
    sw2 = din("sw2", [N_SLAB, 128, PER_SLAB, D], F16)
    b1o = din("b1o", [128, 2 * MF], F32)
    b1b = din("b1b", [128, 2 * MF], F32)
    sb1 = din("sb1", [128, 2 * MF], F32)
    b2o = din("b2o", [128, D], F32)
    b2b = din("b2b", [128, D], F32)
    sb2 = din("sb2", [128, D], F32)
    cwd = din("cw", [128, G], F32)                     # combine wt per col
    scat = din("scat", [128, G], I32)                  # row in half's a2a_in
    g0i = din("g0i", [128, n_to], I32)                 # abs row in a2a_out
    g1i = din("g1i", [128, n_to], I32)
    out = nc.dram_tensor("out", [TO, D], F32, kind="ExternalOutput").ap()

    with tile.TileContext(nc) as tc:
        with contextlib.ExitStack() as ctx:
            sbuf = ctx.enter_context(tc.tile_pool(name="sbuf", bufs=1))
            psA = ctx.enter_context(tc.tile_pool(name="psA", bufs=3,
                                                 space="PSUM"))
            psB = ctx.enter_context(tc.tile_pool(name="psB", bufs=2,
                                                 space="PSUM"))
            dpool = ctx.enter_context(tc.tile_pool(name="dram", bufs=1,
                                                   space="DRAM"))

            a2a_in0 = dpool.tile([rows_h, D], F16)
            a2a_in1 = dpool.tile([rows_h, D], F16)
            a2a_out = dpool.tile([2 * rows_h, D], F16)
            a2a_ins = [a2a_in0, a2a_in1]

            # PE warmup: trip the HAM activity window during the input
            # DMAs so the first real matmuls run at full clock. Fed from a
            # memset tile so no DMA gates it.
            wu = sbuf.tile([128, 512], F16, tag="wu", name="wu", bufs=1)
            nc.vector.memset(wu[:], 1.0)
            for _ in range(34):
                ps_w = psA.tile([128, 512], F32, space="PSUM", tag="ps_a",
                                name="ps_a", bufs=3)
                nc.tensor.matmul(out=ps_w[:1, :], lhsT=wu[:, :1],
                                 rhs=wu[:], start=True, stop=True)

            # biases + index grids (resident; off the sync queue, which is
            # reserved for the bulk x/w stream)
            b1ot = sbuf.tile([128, 2 * MF], F32, tag="b1ot", name="b1ot",
                             bufs=1)
            b1bt = sbuf.tile([128, 2 * MF], F32, tag="b1bt", name="b1bt",
                             bufs=1)
            sb1t = sbuf.tile([128, 2 * MF], F32, tag="sb1t", name="sb1t",
                             bufs=1)
            cwt = sbuf.tile([128, G], F32, tag="cwt", name="cwt", bufs=1)
            sct = sbuf.tile([128, G], I32, tag="sct", name="sct", bufs=1)
            b2ot = sbuf.tile([128, D], F32, tag="b2ot", name="b2ot", bufs=1)
            b2bt = sbuf.tile([128, D], F32, tag="b2bt", name="b2bt", bufs=1)
            sb2t = sbuf.tile([128, D], F32, tag="sb2t", name="sb2t", bufs=1)
            nc.scalar.dma_start(out=b1ot[:], in_=b1o[:])
            nc.gpsimd.dma_start(out=b1bt[:], in_=b1b[:])
            nc.gpsimd.dma_start(out=cwt[:], in_=cwd[:])
            nc.gpsimd.dma_start(out=sct[:], in_=scat[:])
            nc.gpsimd.dma_start(out=sb1t[:], in_=sb1[:])

            def mk_fin(cbase, y_tiles, a2a_in, yh_box):
                def fin_tile(q, t, rows, d, ds, last):
                    # *combine weight (scalar engine), scatter to a2a buf
                    if q != N_SLAB - 1:
                        return
                    col = cbase + t
                    if d == 0:
                        yh_box[0] = sbuf.tile([128, D], F16, tag="yh",
                                              name="yh", bufs=2)
                    yh = yh_box[0]
                    nc.scalar.activation(yh[:rows, ds], y_tiles[t][:rows, ds],
                                         mybir.ActivationFunctionType.Copy,
                                         scale=cwt[:rows, col:col + 1])
                    if last:
                        nc.gpsimd.indirect_dma_start(
                            out=a2a_in[:],
                            out_offset=bass.IndirectOffsetOnAxis(
                                ap=sct[:rows, col:col + 1], axis=0),
                            in_=yh[:rows, :],
                            in_offset=None,
                            bounds_check=rows_h - 1,
                            oob_is_err=False,
                        )
                return fin_tile

            # ---------------- routed expert (2 half-passes over tokens) ----
            x_eng3 = [(nc.sync, nc.scalar), (nc.gpsimd, nc.sync),
                      (nc.scalar, nc.gpsimd)]
            for h in range(2):
                y_tiles = [sbuf.tile([128, D], F32, tag=f"ya{t}",
                                     name=f"ya{t}", bufs=1)
                           for t in range(n_thR)]
                own = _Group([xg0, xg1][h], hR, w1o, w2o, b1ot, b2ot,
                             y_tiles,
                             mk_fin(h * n_thR, y_tiles, a2a_ins[h], [None]),
                             nc.sync, (nc.sync, nc.scalar))
                grps = [own]
                if h == 1:
                    yb_tiles = [sbuf.tile([128, D], F32, tag=f"yb{t}",
                                          name=f"yb{t}", bufs=1)
                                for t in range(n_thB)]
                    grps.append(_Group(
                        xgb, c_b, w1b, w2b, b1bt, b2bt, yb_tiles,
                        mk_fin(2 * n_thR, yb_tiles, a2a_ins[1], [None]),
                        nc.scalar, (nc.gpsimd, nc.gpsimd)))
                pre = ((lambda: nc.gpsimd.dma_start(out=b2ot[:], in_=b2o[:]))
                       if h == 0 else
                       (lambda: nc.gpsimd.dma_start(out=b2bt[:], in_=b2b[:])))
                _ffn_phase(nc, sbuf, psA, psB, grps, x_eng3, pre_loads=pre)
                # dispatch this half back to the combiner cores
                nc.gpsimd.collective_compute(
                    "AllToAll",
                    mybir.AluOpType.bypass,
                    replica_groups=[list(range(NCORES))],
                    ins=[a2a_ins[h][:].opt()],
                    outs=[a2a_out[h * rows_h:(h + 1) * rows_h, :].opt()],
                )

            # ---------------- shared expert on owned tokens (overlaps) ----
            i0 = sbuf.tile([128, n_to], I32, tag="i0", name="i0", bufs=1)
            i1 = sbuf.tile([128, n_to], I32, tag="i1", name="i1", bufs=1)
            nc.scalar.dma_start(out=i0[:], in_=g0i[:])
            nc.scalar.dma_start(out=i1[:], in_=g1i[:])
            # gathers of routed contributions are emitted via pre_loads so
            # the gpsimd queue serves the shared phase's x pieces first;
            # they overlap shared compute (waiting on both AllToAlls via
            # the a2a_out dependency)
            r0s, r1s = [], []

            def shared_pre():
                nc.gpsimd.dma_start(out=sb2t[:], in_=sb2[:])
                for t in range(n_to):
                    r0 = sbuf.tile([128, D], F16, tag="r0", name="r0", bufs=4)
                    r1 = sbuf.tile([128, D], F16, tag="r1", name="r1", bufs=4)
                    nc.gpsimd.indirect_dma_start(
                        out=r0[:], out_offset=None, in_=a2a_out[:],
                        in_offset=bass.IndirectOffsetOnAxis(
                            ap=i0[:, t:t + 1], axis=0))
                    nc.gpsimd.indirect_dma_start(
                        out=r1[:], out_offset=None, in_=a2a_out[:],
                        in_offset=bass.IndirectOffsetOnAxis(
                            ap=i1[:, t:t + 1], axis=0))
                    r0s.append(r0)
                    r1s.append(r1)

            ys_tiles = [sbuf.tile([128, D], F32, tag=f"ya{t}", name=f"ya{t}",
                                  bufs=1)
                        for t in range(n_to)]

            def comb_tile(q, t, rows, d, ds, last):
                # routed contributions fold in one slab early (commutative);
                # after the final slab only a half-tile store trails the
                # last matmul
                if q == N_SLAB - 2:
                    yt = ys_tiles[t]
                    nc.vector.tensor_add(yt[:, ds], yt[:, ds], r0s[t][:, ds])
                    nc.vector.tensor_add(yt[:, ds], yt[:, ds], r1s[t][:, ds])
                elif q == N_SLAB - 1:
                    tr = slice(t * 128, (t + 1) * 128)
                    if t == n_to - 1 and last:
                        h0 = slice(ds.start, ds.start + 256)
                        h1 = slice(ds.start + 256, ds.stop)
                        nc.sync.dma_start(out=out[tr, h0],
                                          in_=ys_tiles[t][:, h0])
                        nc.scalar.dma_start(out=out[tr, h1],
                                            in_=ys_tiles[t][:, h1])
                    else:
                        eng = nc.sync if (t + d) % 2 == 0 else nc.scalar
                        eng.dma_start(out=out[tr, ds],
                                      in_=ys_tiles[t][:, ds])

            shared_g = _Group(xs, TO, sw1, sw2, sb1t, sb2t, ys_tiles,
                              comb_tile, nc.sync, (nc.sync, nc.scalar))
            _ffn_phase(nc, sbuf, psA, psB, [shared_g], x_eng3,
                       pre_loads=shared_pre)

    nc.compile()
    _nc_cache[key] = nc
    return nc


def _route(x, gate_w, gate_b):
    """Host gate: softmax top-2 (float64 for stable ordering)."""
    logits = (x.astype(np.float64) @ gate_w.astype(np.float64)
              + gate_b.astype(np.float64))
    m = logits.max(axis=-1, keepdims=True)
    p = np.exp(logits - m)
    p /= p.sum(axis=-1, keepdims=True)
    order = np.argsort(-p, axis=-1)
    idx = order[:, :TOPK]                      # [T, 2]
    wts = np.take_along_axis(p, idx, axis=-1)  # [T, 2]
    return idx, wts.astype(np.float32)


def _solve_caps(loads):
    """min R+B s.t. sum_e ceil((c_e-R)+ / B) <= NCORES, caps mult of 4."""
    best = None
    for R in range(1024, max(loads) + 4, 4):
        ovf = [c - R for c in loads if c > R]
        if not ovf:
            if best is None or R + 8 < best[0] + best[1]:
                best = (R, 8)
            break
        B = 4
        while B <= R:
            if sum(-(-o // B) for o in ovf) <= NCORES:
                break
            B += 4
        else:
            continue
        if best is None or R + B < best[0] + best[1]:
            best = (R, B)
    return best


def _tile_w1(w):      # [D, 2F] -> [MF, 128, 2, KD, 128]
    v = np.asarray(w, np.float16).reshape(KD, 128, 2 * MF, 128)
    s = np.stack([v[:, :, :MF, :], v[:, :, MF:, :]], axis=0)  # [j,k,p,m,c]
    return np.ascontiguousarray(s.transpose(3, 2, 0, 1, 4))


def _tile_w2(w):      # [F, D] -> [N_SLAB, 128, PER_SLAB, D]
    v = np.asarray(w, np.float16).reshape(N_SLAB, PER_SLAB, 128, D)
    return np.ascontiguousarray(v.transpose(0, 2, 1, 3))


def _col_bias(b):     # [2F] -> [128, 2*MF]
    return np.ascontiguousarray(
        np.asarray(b, np.float32).reshape(2 * MF, 128).T)


def _xT_blocks(xr, c_cap):
    """[C, D] fp16 -> per-chunk contiguous blocks [n_ch, 128, KD, 512]."""
    ct = np.zeros((128, KD, c_cap), np.float16)
    n = len(xr)
    if n:
        ct[:, :, :n] = xr.T.reshape(KD, 128, n).transpose(1, 0, 2)
    chunks = _chunk_slices(c_cap)
    blk = np.zeros((len(chunks), 128, KD, 512), np.float16)
    for ci, (cs, cw) in enumerate(chunks):
        blk[ci, :, :, :cw] = ct[:, :, cs:cs + cw]
    return blk


def kernel(x, gate_w, gate_b, shared_w1, shared_b1, shared_w2, shared_b2,
           routed_w1, routed_b1, routed_w2, routed_b2):
    x = np.asarray(x, dtype=np.float32)
    topk_idx, topk_w = _route(x, np.asarray(gate_w), np.asarray(gate_b))

    owner = np.arange(T) // TO                 # owning core per token

    # per-expert dispatch lists, ordered by (owner, token)
    tok_lists, wt_lists = [], []
    for e in range(E):
        sel = (topk_idx == e)                  # [T, 2]
        tsel = np.nonzero(sel.any(axis=1))[0]  # ascending => owner-sorted
        k_of = sel[tsel, 1].astype(np.int64)   # slot (experts distinct)
        w_of = topk_w[tsel, :][np.arange(len(tsel)), k_of]
        tok_lists.append(tsel)
        wt_lists.append(w_of)

    loads = [len(t) for t in tok_lists]
    c_r, c_b = _solve_caps(loads)
    hR = c_r // 2
    n_thR = _n_tiles(hR)
    n_thB = _n_tiles(c_b)
    G = 2 * n_thR + n_thB

    # borrowed pieces: overflow of hot experts, one piece per core
    empty_t = np.zeros(0, np.int64)
    empty_w = np.zeros(0, np.float32)
    pieces = []
    for e in range(E):
        for s in range(min(loads[e], c_r), loads[e], c_b):
            pieces.append((e, tok_lists[e][s:s + c_b],
                           wt_lists[e][s:s + c_b]))
    assert len(pieces) <= NCORES, (loads, c_r, c_b)
    # per-core groups: [half0 own, half1 own, half1 borrowed]
    core_groups = []
    for c in range(NCORES):
        bw = pieces[c] if c < len(pieces) else (c, empty_t, empty_w)
        own_t, own_w = tok_lists[c], wt_lists[c]
        core_groups.append([(c, own_t[:hR], own_w[:hR]),
                            (c, own_t[hR:c_r], own_w[hR:c_r]), bw])

    # a2a row cap: per (src core, half, dest core) group sizes; half 1
    # packs own + borrowed tokens jointly per dest
    p0 = 1
    for c in range(NCORES):
        g0, g1, gb = core_groups[c]
        cnt0 = np.bincount(owner[g0[1]], minlength=NCORES)
        cnt1 = (np.bincount(owner[g1[1]], minlength=NCORES)
                + np.bincount(owner[gb[1]], minlength=NCORES))
        p0 = max(p0, int(cnt0.max()), int(cnt1.max()))
    p0 = -(-p0 // 8) * 8
    rows_h = NCORES * p0

    nc = _build(c_r, c_b, p0)

    # host-side layouts (fp16 compute dtype)
    w1r = np.asarray(routed_w1, np.float16)              # [E, D, 2F]
    w2r = np.asarray(routed_w2, np.float16)              # [E, F, D]
    b1r = np.asarray(routed_b1)                          # [E, 2F]
    b2r = np.asarray(routed_b2, np.float32)              # [E, D]
    xr = x.astype(np.float16)                            # [T, D]

    sw1_t = _tile_w1(np.asarray(shared_w1, np.float16)[0])
    sw2_t = _tile_w2(np.asarray(shared_w2, np.float16)[0])
    sb1_t = _col_bias(np.asarray(shared_b1)[0])

    # absolute a2a_out row for each (token, slot); combine-wt/scatter grids.
    # Grid columns: half0 own tiles [0, n_thR), half1 own [n_thR, 2*n_thR),
    # half1 borrowed [2*n_thR, G). Scatter row (within the half's a2a_in)
    # = dest*p0 + pos, pos running jointly over (own, borrowed) per dest.
    slot_rows = np.zeros((T, TOPK), np.int64)
    grids_cw = [np.zeros((128, G), np.float32) for _ in range(NCORES)]
    grids_sc = [np.full((128, G), 2**31 - 1, np.int32) for _ in range(NCORES)]
    for c in range(NCORES):
        for h in range(2):
            if h == 0:
                parts = [(0, core_groups[c][0])]
            else:
                parts = [(n_thR, core_groups[c][1]),
                         (2 * n_thR, core_groups[c][2])]
            next_pos = np.zeros(NCORES, np.int64)
            for cbase, (e, toks, wts) in parts:
                if len(toks) == 0:
                    continue
                own = owner[toks]
                pos = np.zeros(len(toks), np.int64)
                for o in range(NCORES):
                    m = own == o
                    pos[m] = next_pos[o] + np.arange(m.sum())
                    next_pos[o] += m.sum()
                jr = np.arange(len(toks)) % 128
                jc = np.arange(len(toks)) // 128
                grids_sc[c][jr, cbase + jc] = own * p0 + pos
                grids_cw[c][jr, cbase + jc] = wts
                abs_rows = h * rows_h + c * p0 + pos
                sel = (topk_idx[toks] == e)
                k_of = sel[:, 1].astype(np.int64)
                slot_rows[toks, k_of] = abs_rows

    in_maps = []
    for c in range(NCORES):
        (_, h0_toks, _), (_, h1_toks, _), (be, b_toks, _) = core_groups[c]

        g0 = np.ascontiguousarray(
            slot_rows[c * TO:(c + 1) * TO, 0].astype(np.int32)
            .reshape(TO // 128, 128).T)
        g1 = np.ascontiguousarray(
            slot_rows[c * TO:(c + 1) * TO, 1].astype(np.int32)
            .reshape(TO // 128, 128).T)

        in_maps.append({
            "xg0": _xT_blocks(xr[h0_toks], hR),
            "xg1": _xT_blocks(xr[h1_toks], hR),
            "xgb": _xT_blocks(xr[b_toks], c_b),
            "xs": _xT_blocks(xr[c * TO:(c + 1) * TO], TO),
            "w1o": _tile_w1(w1r[c]), "w2o": _tile_w2(w2r[c]),
            "w1b": _tile_w1(w1r[be]), "w2b": _tile_w2(w2r[be]),
            "sw1": sw1_t, "sw2": sw2_t,
            "b1o": _col_bias(b1r[c]), "b1b": _col_bias(b1r[be]),
            "sb1": sb1_t,
            "b2o": np.ascontiguousarray(np.broadcast_to(b2r[c], (128, D))),
            "b2b": np.ascontiguousarray(np.broadcast_to(b2r[be], (128, D))),
            "sb2": np.ascontiguousarray(np.broadcast_to(
                np.asarray(shared_b2, np.float32)[0], (128, D))),
            "cw": grids_cw[c], "scat": grids_sc[c], "g0i": g0, "g1i": g1,
        })

    res = run_bass_kernel_spmd(nc, in_maps, list(range(NCORES)))
    return np.concatenate([res.results[c]["out"] for c in range(NCORES)],
                          axis=0)


# revision 15
# speedup vs baseline: 1.0242x; 1.0242x over previous
"""MoE (top-2 of 8 experts + 1 shared expert, SwiGLU FFN) on 8 TRN2 NeuronCores.

Strategy (expert-parallel + load-balanced overflow, per the sharding hint):
  - Host computes the (tiny) gate: softmax top-2 over E=8 for T=8192 tokens,
    and from it the dispatch. >99.9% of FLOPs (the FFNs) run on device.
  - Load balancing: expert loads vary (~1932..2182 for T=8192); SPMD padding
    to the max would cost ~5%. Instead each core runs its own expert's first
    R tokens (R ~ 1980, two half-passes) plus a small "borrowed" group
    (cap cB ~ 104) of overflow tokens from some hot expert, with that
    expert's weights supplied per-core. (R, cB) solve
    min R+cB s.t. sum_e ceil((c_e-R)+ / cB) <= 8, bringing per-core routed
    work from max_e c_e (~2184) down to ~mean+2% (~2084).
  - The borrowed group rides INSIDE half-pass 1 as a second token group:
    its weight stream (w1b/w2b, 24MB) amortizes over the whole half-pass.
    A standalone borrowed pass would need >500GB/s of weight bandwidth.
  - Each half-pass scales rows by the gate weight and scatters rows into an
    AllToAll dispatch buffer laid out by destination core; each half's
    AllToAll fires when ready, overlapping remaining compute. Each core also
    runs the shared expert on its own T/8 token slice.
  - Combine on device: out[t] = shared(t) + contrib0(t) + contrib1(t), where
    contribs are indirect-gathered from the a2a output segments (host
    precomputes absolute rows; placement is fully host-controlled).

Compute dtype is fp16 (PSUM accumulation fp32). The PE clock is power-capped
at ~1.95 GHz sustained (GPIO throttle k=13/16), so the kernel is tuned to
keep the MM stream dense: all DMAs are large partition-major transfers
(weights are host-tiled so each f-chunk / w2-slab is a single DMA), biases
are folded into the PSUM->SBUF drain, the per-row gate scale runs on the
scalar engine, and the finalize/combine work is interleaved per-tile with
the last stage-2 slab so nothing serializes after the final matmul. At
phase start, x chunks >=1 and the slab-0 w2 load are emitted behind the
first weight tiles so the first matmuls aren't queued behind bulk DMAs.
"""
import contextlib

import numpy as np

import concourse.bass as bass
import concourse.tile as tile
from concourse import bacc, mybir
from concourse.bass_utils import run_bass_kernel_spmd

# problem shape (hardcoded per contract)
T = 8192
D = 1024
F = 4096
E = 8
TOPK = 2
NCORES = 8
TO = T // NCORES          # tokens owned per core

F32 = mybir.dt.float32
F16 = mybir.dt.float16
I32 = mybir.dt.int32

MF = 2 * F // 128 // 2    # 32 f-chunks (a-half; b-half is mp+MF)
N_SLAB = 4
PER_SLAB = MF // N_SLAB   # 8 f-chunks per slab
KD = D // 128             # 8 contraction chunks for stage 1

_nc_cache: dict[tuple, object] = {}


def _chunk_slices(c_len):
    """Moving-dim chunks of <=512, each >=256 so LDWEIGHTS stays hidden.

    Largest chunk first: it gates the first matmul of a pass, and the
    extra matmul runway it provides covers the smaller chunks' transfers.
    """
    out = []
    rem = c_len
    while rem > 0:
        if rem <= 512:
            w = rem
        elif rem >= 768:
            w = 512
        else:
            w = rem - 256
        out.append(w)
        rem -= w
    out.sort(reverse=True)
    widths = []
    pos = 0
    for w in out:
        widths.append((pos, w))
        pos += w
    return widths


def _n_tiles(c_len):
    return -(-c_len // 128)


class _Group:
    """One token group within an FFN pass: its own x, weights, biases,
    output tiles and finalize callback."""

    def __init__(self, x_src, c_len, w1d, w2d, b1t, b2t, y_tiles, on_tile_d,
                 w1_eng, w2_engs):
        self.x_src = x_src
        self.c_len = c_len
        self.w1d = w1d
        self.w2d = w2d
        self.b1t = b1t
        self.b2t = b2t
        self.y_tiles = y_tiles
        self.on_tile_d = on_tile_d
        self.w1_eng = w1_eng
        self.w2_engs = w2_engs
        self.chunks = _chunk_slices(c_len)
        self.xk = []
        self.g_tiles = []
        self.w2t = None


def _ffn_phase(nc, sbuf, psA, psB, groups, x_eng, pre_loads=None):
    """Emit one SwiGLU FFN pass over one or more token groups.

    Per group: x_src is per-chunk DRAM APs [128, KD, 512]; w1d is DRAM
    [MF, 128, 2, KD, 128] fp16 (host-tiled; [...,0,...]=a, 1=b); w2d is
    DRAM [N_SLAB, 128, PER_SLAB, D] fp16; b1t SBUF [128, 2*MF] f32 (col m =
    bias for a-chunk m / b-chunk m-MF); b2t SBUF [128, D] f32 added once
    into y at slab 0; y_tiles receive FFN output + bias2;
    on_tile_d(q, t, rows, d, ds, last) runs after each stage-2 add.

    Every x chunk / k-half and each w2 half lives in its own tile so the
    DMAs carry no false whole-tile dependencies and transfer concurrently.
    Only the first chunk's x is loaded ahead of the first weight tile; the
    rest are deferred so the first matmul isn't queued behind bulk DMAs.
    """
    KH = KD // 2
    ei = 0
    deferred_x = []
    for gi, g in enumerate(groups):
        g.xk = []
        for ci, (cs, cw) in enumerate(g.chunks):
            xa = sbuf.tile([128, KH, 512], F16, tag=f"x{gi}c{ci}a",
                           name=f"x{gi}c{ci}a", bufs=2 if gi == 0 else 1)
            xb = sbuf.tile([128, KH, 512], F16, tag=f"x{gi}c{ci}b",
                           name=f"x{gi}c{ci}b", bufs=2 if gi == 0 else 1)
            e0, e1 = x_eng[ei % len(x_eng)]
            ei += 1
            if gi == 0 and ci == 0:
                e0.dma_start(out=xa[:], in_=g.x_src[ci][:, 0:KH])
                e1.dma_start(out=xb[:], in_=g.x_src[ci][:, KH:KD])
            else:
                deferred_x.append((e0, xa, e1, xb, g.x_src[ci]))
            g.xk.append((xa, xb))
    if pre_loads is not None:
        pre_loads()

    for q in range(N_SLAB):
        for g in groups:
            g.g_tiles = []
        for fi in range(PER_SLAB):
            mp = q * PER_SLAB + fi
            for gi, g in enumerate(groups):
                w1t = sbuf.tile([128, 2, KD, 128], F16, tag=f"w1t{gi}",
                                name=f"w1t{gi}", bufs=8 if gi == 0 else 4)
                g.w1_eng.dma_start(out=w1t[:], in_=g.w1d[mp])
                if q == 0 and fi == 0 and gi == 0:
                    # x chunks beyond the first ride behind the first
                    # weight tile in queue order
                    for (e0, xa, e1, xb, src) in deferred_x:
                        e0.dma_start(out=xa[:], in_=src[:, 0:KH])
                        e1.dma_start(out=xb[:], in_=src[:, KH:KD])
                g_t = sbuf.tile([128, g.c_len], F16, tag=f"g{gi}_{fi}",
                                name=f"g{gi}_{fi}", bufs=1)
                for ci, (cs, cw) in enumerate(g.chunks):
                    ps_a = psA.tile([128, 512], F32, space="PSUM", tag="ps_a",
                                    name="ps_a", bufs=3)
                    ps_b = psA.tile([128, 512], F32, space="PSUM", tag="ps_b",
                                    name="ps_b", bufs=3)
                    for k in range(KD):
                        rhs = g.xk[ci][k // KH][:, k % KH, :cw]
                        nc.tensor.matmul(out=ps_a[:, :cw],
                                         lhsT=w1t[:, 0, k, :], rhs=rhs,
                                         start=(k == 0), stop=(k == KD - 1))
                    for k in range(KD):
                        rhs = g.xk[ci][k // KH][:, k % KH, :cw]
                        nc.tensor.matmul(out=ps_b[:, :cw],
                                         lhsT=w1t[:, 1, k, :], rhs=rhs,
                                         start=(k == 0), stop=(k == KD - 1))
                    t_a = sbuf.tile([128, 512], F16, tag="t_a", name="t_a",
                                    bufs=6)
                    t_b = sbuf.tile([128, 512], F16, tag="t_b", name="t_b",
                                    bufs=6)
                    nc.scalar.activation(t_a[:, :cw], ps_a[:, :cw],
                                         mybir.ActivationFunctionType.Silu,
                                         bias=g.b1t[:, mp:mp + 1])
                    nc.scalar.activation(
                        t_b[:, :cw], ps_b[:, :cw],
                        mybir.ActivationFunctionType.Identity,
                        bias=g.b1t[:, mp + MF:mp + MF + 1])
                    nc.vector.tensor_mul(g_t[:, cs:cs + cw], t_a[:, :cw],
                                         t_b[:, :cw])
                g.g_tiles.append(g_t)
            # stage-2 weights for this slab (streamed during stage-1).
            # Slab 0's load is emitted mid-pass so it doesn't sit in front
            # of the first weight tiles in the queue at phase start.
            if fi == (3 if q == 0 else 0):
                for gi, g in enumerate(groups):
                    w2ta = sbuf.tile([128, PER_SLAB // 2, D], F16,
                                     tag=f"w2{gi}a", name=f"w2{gi}a", bufs=1)
                    w2tb = sbuf.tile([128, PER_SLAB // 2, D], F16,
                                     tag=f"w2{gi}b", name=f"w2{gi}b", bufs=1)
                    g.w2_engs[0].dma_start(out=w2ta[:],
                                           in_=g.w2d[q][:, 0:PER_SLAB // 2])
                    g.w2_engs[1].dma_start(
                        out=w2tb[:], in_=g.w2d[q][:, PER_SLAB // 2:PER_SLAB])
                    g.w2t = (w2ta, w2tb)
        # stage-2 partial: y (+)= g_slab.T @ w2_slab
        for g in groups:
            n_t = _n_tiles(g.c_len)
            w2ta, w2tb = g.w2t
            for t in range(n_t):
                rows = min(128, g.c_len - t * 128)
                ts = slice(t * 128, t * 128 + rows)
                for d in range(D // 512):
                    ds = slice(d * 512, (d + 1) * 512)
                    ps_y = psB.tile([128, 512], F32, space="PSUM",
                                    tag="ps_y", name="ps_y", bufs=2)
                    for fi in range(PER_SLAB):
                        w2s = w2ta if fi < PER_SLAB // 2 else w2tb
                        nc.tensor.matmul(
                            out=ps_y[:rows, :], lhsT=g.g_tiles[fi][:, ts],
                            rhs=w2s[:, fi % (PER_SLAB // 2), ds],
                            start=(fi == 0), stop=(fi == PER_SLAB - 1))
                    yt = g.y_tiles[t]
                    if q == 0:
                        nc.vector.tensor_add(yt[:rows, ds], ps_y[:rows, :],
                                             g.b2t[:rows, ds])
                    else:
                        nc.vector.tensor_add(yt[:rows, ds], yt[:rows, ds],
                                             ps_y[:rows, :])
                    g.on_tile_d(q, t, rows, d, ds, d == D // 512 - 1)


def _build(c_r, c_b, p0):
    """SPMD program: half-pass 0 (own expert, c_r/2 tokens), half-pass 1
    (own c_r/2 + borrowed c_b as a second group), shared pass. Each half
    scatters into its own AllToAll buffer; the shared pass gathers +
    combines + stores."""
    key = (c_r, c_b, p0)
    if key in _nc_cache:
        return _nc_cache[key]

    nc = bacc.Bacc("TRN2", target_bir_lowering=False, debug=False,
                   num_devices=NCORES)

    def din(name, shape, dt):
        return nc.dram_tensor(name, shape, dt, kind="ExternalInput").ap()

    hR = c_r // 2
    n_thR = _n_tiles(hR)                               # y tiles per R half
    n_thB = _n_tiles(c_b)
    n_to = TO // 128
    G = 2 * n_thR + n_thB                              # cw/scat grid columns
    rows_h = NCORES * p0                               # rows per half buffer

    n_chR = len(_chunk_slices(hR))
    n_chB = len(_chunk_slices(c_b))
    n_cs = len(_chunk_slices(TO))
    # gathered/owned tokens^T, one contiguous 512-wide block per chunk
    xg0 = din("xg0", [n_chR, 128, KD, 512], F16)
    xg1 = din("xg1", [n_chR, 128, KD, 512], F16)
    xgb = din("xgb", [n_chB, 128, KD, 512], F16)
    xs = din("xs", [n_cs, 128, KD, 512], F16)
    w1o = din("w1o", [MF, 128, 2, KD, 128], F16)
    w2o = din("w2o", [N_SLAB, 128, PER_SLAB, D], F16)
    w1b = din("w1b", [MF, 128, 2, KD, 128], F16)
    w2b = din("w2b", [N_SLAB, 128, PER_SLAB, D], F16)
    sw1 = din("sw1", [MF, 128, 2, KD, 128], F16)
    sw2 = din("sw2", [N_SLAB, 128, PER_SLAB, D], F16)
    b1o = din("b1o", [128, 2 * MF], F32)
    b1b = din("b1b", [128, 2 * MF], F32)
    sb1 = din("sb1", [128, 2 * MF], F32)
    b2o = din("b2o", [128, D], F32)
    b2b = din("b2b", [128, D], F32)
    sb2 = din("sb2", [128, D], F32)
    cwd = din("cw", [128, G], F32)                     # combine wt per col
    scat = din("scat", [128, G], I32)                  # row in half's a2a_in
    g0i = din("g0i", [128, n_to], I32)                 # abs row in a2a_out
    g1i = din("g1i", [128, n_to], I32)
    out = nc.dram_tensor("out", [TO, D], F16, kind="ExternalOutput").ap()

    with tile.TileContext(nc) as tc:
        with contextlib.ExitStack() as ctx:
            sbuf = ctx.enter_context(tc.tile_pool(name="sbuf", bufs=1))
            psA = ctx.enter_context(tc.tile_pool(name="psA", bufs=3,
                                                 space="PSUM"))
            psB = ctx.enter_context(tc.tile_pool(name="psB", bufs=2,
                                                 space="PSUM"))
            dpool = ctx.enter_context(tc.tile_pool(name="dram", bufs=1,
                                                   space="DRAM"))

            a2a_in0 = dpool.tile([rows_h, D], F16)
            a2a_in1 = dpool.tile([rows_h, D], F16)
            a2a_out = dpool.tile([2 * rows_h, D], F16)
            a2a_ins = [a2a_in0, a2a_in1]

            # PE warmup: trip the HAM activity window during the input
            # DMAs so the first real matmuls run at full clock. Fed from a
            # memset tile so no DMA gates it.
            wu = sbuf.tile([128, 512], F16, tag="wu", name="wu", bufs=1)
            nc.vector.memset(wu[:], 1.0)
            for _ in range(34):
                ps_w = psA.tile([128, 512], F32, space="PSUM", tag="ps_a",
                                name="ps_a", bufs=3)
                nc.tensor.matmul(out=ps_w[:1, :], lhsT=wu[:, :1],
                                 rhs=wu[:], start=True, stop=True)

            # biases + index grids (resident; off the sync queue, which is
            # reserved for the bulk x/w stream)
            b1ot = sbuf.tile([128, 2 * MF], F32, tag="b1ot", name="b1ot",
                             bufs=1)
            b1bt = sbuf.tile([128, 2 * MF], F32, tag="b1bt", name="b1bt",
                             bufs=1)
            sb1t = sbuf.tile([128, 2 * MF], F32, tag="sb1t", name="sb1t",
                             bufs=1)
            cwt = sbuf.tile([128, G], F32, tag="cwt", name="cwt", bufs=1)
            sct = sbuf.tile([128, G], I32, tag="sct", name="sct", bufs=1)
            b2ot = sbuf.tile([128, D], F32, tag="b2ot", name="b2ot", bufs=1)
            b2bt = sbuf.tile([128, D], F32, tag="b2bt", name="b2bt", bufs=1)
            sb2t = sbuf.tile([128, D], F32, tag="sb2t", name="sb2t", bufs=1)
            nc.scalar.dma_start(out=b1ot[:], in_=b1o[:])
            nc.gpsimd.dma_start(out=b1bt[:], in_=b1b[:])
            nc.gpsimd.dma_start(out=cwt[:], in_=cwd[:])
            nc.gpsimd.dma_start(out=sct[:], in_=scat[:])
            nc.gpsimd.dma_start(out=sb1t[:], in_=sb1[:])

            def mk_fin(cbase, y_tiles, a2a_in, yh_box):
                def fin_tile(q, t, rows, d, ds, last):
                    # *combine weight (scalar engine), scatter to a2a buf
                    if q != N_SLAB - 1:
                        return
                    col = cbase + t
                    if d == 0:
                        yh_box[0] = sbuf.tile([128, D], F16, tag="yh",
                                              name="yh", bufs=2)
                    yh = yh_box[0]
                    nc.scalar.activation(yh[:rows, ds], y_tiles[t][:rows, ds],
                                         mybir.ActivationFunctionType.Copy,
                                         scale=cwt[:rows, col:col + 1])
                    if last:
                        nc.gpsimd.indirect_dma_start(
                            out=a2a_in[:],
                            out_offset=bass.IndirectOffsetOnAxis(
                                ap=sct[:rows, col:col + 1], axis=0),
                            in_=yh[:rows, :],
                            in_offset=None,
                            bounds_check=rows_h - 1,
                            oob_is_err=False,
                        )
                return fin_tile

            # ---------------- routed expert (2 half-passes over tokens) ----
            x_eng3 = [(nc.sync, nc.scalar), (nc.gpsimd, nc.sync),
                      (nc.scalar, nc.gpsimd)]
            for h in range(2):
                y_tiles = [sbuf.tile([128, D], F16, tag=f"ya{t}",
                                     name=f"ya{t}", bufs=1)
                           for t in range(n_thR)]
                own = _Group([xg0, xg1][h], hR, w1o, w2o, b1ot, b2ot,
                             y_tiles,
                             mk_fin(h * n_thR, y_tiles, a2a_ins[h], [None]),
                             nc.sync, (nc.sync, nc.scalar))
                grps = [own]
                if h == 1:
                    yb_tiles = [sbuf.tile([128, D], F16, tag=f"yb{t}",
                                          name=f"yb{t}", bufs=1)
                                for t in range(n_thB)]
                    grps.append(_Group(
                        xgb, c_b, w1b, w2b, b1bt, b2bt, yb_tiles,
                        mk_fin(2 * n_thR, yb_tiles, a2a_ins[1], [None]),
                        nc.sync, (nc.gpsimd, nc.gpsimd)))
                pre = ((lambda: nc.gpsimd.dma_start(out=b2ot[:], in_=b2o[:]))
                       if h == 0 else
                       (lambda: nc.gpsimd.dma_start(out=b2bt[:], in_=b2b[:])))
                _ffn_phase(nc, sbuf, psA, psB, grps, x_eng3, pre_loads=pre)
                # dispatch this half back to the combiner cores
                nc.gpsimd.collective_compute(
                    "AllToAll",
                    mybir.AluOpType.bypass,
                    replica_groups=[list(range(NCORES))],
                    ins=[a2a_ins[h][:].opt()],
                    outs=[a2a_out[h * rows_h:(h + 1) * rows_h, :].opt()],
                )

            # ---------------- shared expert on owned tokens (overlaps) ----
            i0 = sbuf.tile([128, n_to], I32, tag="i0", name="i0", bufs=1)
            i1 = sbuf.tile([128, n_to], I32, tag="i1", name="i1", bufs=1)
            nc.scalar.dma_start(out=i0[:], in_=g0i[:])
            nc.scalar.dma_start(out=i1[:], in_=g1i[:])
            # gathers of routed contributions are emitted via pre_loads so
            # the gpsimd queue serves the shared phase's x pieces first;
            # they overlap shared compute (waiting on both AllToAlls via
            # the a2a_out dependency)
            r0s, r1s = [], []

            def shared_pre():
                nc.gpsimd.dma_start(out=sb2t[:], in_=sb2[:])
                for t in range(n_to):
                    r0 = sbuf.tile([128, D], F16, tag="r0", name="r0", bufs=4)
                    r1 = sbuf.tile([128, D], F16, tag="r1", name="r1", bufs=4)
                    nc.gpsimd.indirect_dma_start(
                        out=r0[:], out_offset=None, in_=a2a_out[:],
                        in_offset=bass.IndirectOffsetOnAxis(
                            ap=i0[:, t:t + 1], axis=0))
                    nc.gpsimd.indirect_dma_start(
                        out=r1[:], out_offset=None, in_=a2a_out[:],
                        in_offset=bass.IndirectOffsetOnAxis(
                            ap=i1[:, t:t + 1], axis=0))
                    r0s.append(r0)
                    r1s.append(r1)

            ys_tiles = [sbuf.tile([128, D], F16, tag=f"ya{t}", name=f"ya{t}",
                                  bufs=1)
                        for t in range(n_to)]

            def comb_tile(q, t, rows, d, ds, last):
                # routed contributions fold in one slab early (commutative);
                # after the final slab only a half-tile store trails the
                # last matmul
                if q == N_SLAB - 2:
                    yt = ys_tiles[t]
                    nc.vector.tensor_add(yt[:, ds], yt[:, ds], r0s[t][:, ds])
                    nc.vector.tensor_add(yt[:, ds], yt[:, ds], r1s[t][:, ds])
                elif q == N_SLAB - 1:
                    tr = slice(t * 128, (t + 1) * 128)
                    if t == n_to - 1 and last:
                        h0 = slice(ds.start, ds.start + 256)
                        h1 = slice(ds.start + 256, ds.stop)
                        nc.sync.dma_start(out=out[tr, h0],
                                          in_=ys_tiles[t][:, h0])
                        nc.scalar.dma_start(out=out[tr, h1],
                                            in_=ys_tiles[t][:, h1])
                    else:
                        eng = nc.sync if (t + d) % 2 == 0 else nc.scalar
                        eng.dma_start(out=out[tr, ds],
                                      in_=ys_tiles[t][:, ds])

            shared_g = _Group(xs, TO, sw1, sw2, sb1t, sb2t, ys_tiles,
                              comb_tile, nc.sync, (nc.sync, nc.scalar))
            _ffn_phase(nc, sbuf, psA, psB, [shared_g], x_eng3,
                       pre_loads=shared_pre)

    nc.compile()
    _nc_cache[key] = nc
    return nc


def _route(x, gate_w, gate_b):
    """Host gate: softmax top-2 (float64 for stable ordering)."""
    logits = (x.astype(np.float64) @ gate_w.astype(np.float64)
              + gate_b.astype(np.float64))
    m = logits.max(axis=-1, keepdims=True)
    p = np.exp(logits - m)
    p /= p.sum(axis=-1, keepdims=True)
    order = np.argsort(-p, axis=-1)
    idx = order[:, :TOPK]                      # [T, 2]
    wts = np.take_along_axis(p, idx, axis=-1)  # [T, 2]
    return idx, wts.astype(np.float32)


def _solve_caps(loads):
    """min R+B s.t. sum_e ceil((c_e-R)+ / B) <= NCORES, caps mult of 4."""
    best = None
    for R in range(1024, max(loads) + 4, 4):
        ovf = [c - R for c in loads if c > R]
        if not ovf:
            if best is None or R + 8 < best[0] + best[1]:
                best = (R, 8)
            break
        B = 4
        while B <= R:
            if sum(-(-o // B) for o in ovf) <= NCORES:
                break
            B += 4
        else:
            continue
        if best is None or R + B < best[0] + best[1]:
            best = (R, B)
    return best


def _tile_w1(w):      # [D, 2F] -> [MF, 128, 2, KD, 128]
    v = np.asarray(w, np.float16).reshape(KD, 128, 2 * MF, 128)
    s = np.stack([v[:, :, :MF, :], v[:, :, MF:, :]], axis=0)  # [j,k,p,m,c]
    return np.ascontiguousarray(s.transpose(3, 2, 0, 1, 4))


def _tile_w2(w):      # [F, D] -> [N_SLAB, 128, PER_SLAB, D]
    v = np.asarray(w, np.float16).reshape(N_SLAB, PER_SLAB, 128, D)
    return np.ascontiguousarray(v.transpose(0, 2, 1, 3))


def _col_bias(b):     # [2F] -> [128, 2*MF]
    return np.ascontiguousarray(
        np.asarray(b, np.float32).reshape(2 * MF, 128).T)


def _xT_blocks(xr, c_cap):
    """[C, D] fp16 -> per-chunk contiguous blocks [n_ch, 128, KD, 512]."""
    ct = np.zeros((128, KD, c_cap), np.float16)
    n = len(xr)
    if n:
        ct[:, :, :n] = xr.T.reshape(KD, 128, n).transpose(1, 0, 2)
    chunks = _chunk_slices(c_cap)
    blk = np.zeros((len(chunks), 128, KD, 512), np.float16)
    for ci, (cs, cw) in enumerate(chunks):
        blk[ci, :, :, :cw] = ct[:, :, cs:cs + cw]
    return blk


def kernel(x, gate_w, gate_b, shared_w1, shared_b1, shared_w2, shared_b2,
           routed_w1, routed_b1, routed_w2, routed_b2):
    x = np.asarray(x, dtype=np.float32)
    topk_idx, topk_w = _route(x, np.asarray(gate_w), np.asarray(gate_b))

    owner = np.arange(T) // TO                 # owning core per token

    # per-expert dispatch lists, ordered by (owner, token)
    tok_lists, wt_lists = [], []
    for e in range(E):
        sel = (topk_idx == e)                  # [T, 2]
        tsel = np.nonzero(sel.any(axis=1))[0]  # ascending => owner-sorted
        k_of = sel[tsel, 1].astype(np.int64)   # slot (experts distinct)
        w_of = topk_w[tsel, :][np.arange(len(tsel)), k_of]
        tok_lists.append(tsel)
        wt_lists.append(w_of)

    loads = [len(t) for t in tok_lists]
    c_r, c_b = _solve_caps(loads)
    hR = c_r // 2
    n_thR = _n_tiles(hR)
    n_thB = _n_tiles(c_b)
    G = 2 * n_thR + n_thB

    # borrowed pieces: overflow of hot experts, one piece per core
    empty_t = np.zeros(0, np.int64)
    empty_w = np.zeros(0, np.float32)
    pieces = []
    for e in range(E):
        for s in range(min(loads[e], c_r), loads[e], c_b):
            pieces.append((e, tok_lists[e][s:s + c_b],
                           wt_lists[e][s:s + c_b]))
    assert len(pieces) <= NCORES, (loads, c_r, c_b)
    # per-core groups: [half0 own, half1 own, half1 borrowed]
    core_groups = []
    for c in range(NCORES):
        bw = pieces[c] if c < len(pieces) else (c, empty_t, empty_w)
        own_t, own_w = tok_lists[c], wt_lists[c]
        core_groups.append([(c, own_t[:hR], own_w[:hR]),
                            (c, own_t[hR:c_r], own_w[hR:c_r]), bw])

    # a2a row cap: per (src core, half, dest core) group sizes; half 1
    # packs own + borrowed tokens jointly per dest
    p0 = 1
    for c in range(NCORES):
        g0, g1, gb = core_groups[c]
        cnt0 = np.bincount(owner[g0[1]], minlength=NCORES)
        cnt1 = (np.bincount(owner[g1[1]], minlength=NCORES)
                + np.bincount(owner[gb[1]], minlength=NCORES))
        p0 = max(p0, int(cnt0.max()), int(cnt1.max()))
    p0 = -(-p0 // 8) * 8
    rows_h = NCORES * p0

    nc = _build(c_r, c_b, p0)

    # host-side layouts (fp16 compute dtype)
    w1r = np.asarray(routed_w1, np.float16)              # [E, D, 2F]
    w2r = np.asarray(routed_w2, np.float16)              # [E, F, D]
    b1r = np.asarray(routed_b1)                          # [E, 2F]
    b2r = np.asarray(routed_b2, np.float32)              # [E, D]
    xr = x.astype(np.float16)                            # [T, D]

    sw1_t = _tile_w1(np.asarray(shared_w1, np.float16)[0])
    sw2_t = _tile_w2(np.asarray(shared_w2, np.float16)[0])
    sb1_t = _col_bias(np.asarray(shared_b1)[0])

    # absolute a2a_out row for each (token, slot); combine-wt/scatter grids.
    # Grid columns: half0 own tiles [0, n_thR), half1 own [n_thR, 2*n_thR),
    # half1 borrowed [2*n_thR, G). Scatter row (within the half's a2a_in)
    # = dest*p0 + pos, pos running jointly over (own, borrowed) per dest.
    slot_rows = np.zeros((T, TOPK), np.int64)
    grids_cw = [np.zeros((128, G), np.float32) for _ in range(NCORES)]
    grids_sc = [np.full((128, G), 2**31 - 1, np.int32) for _ in range(NCORES)]
    for c in range(NCORES):
        for h in range(2):
            if h == 0:
                parts = [(0, core_groups[c][0])]
            else:
                parts = [(n_thR, core_groups[c][1]),
                         (2 * n_thR, core_groups[c][2])]
            next_pos = np.zeros(NCORES, np.int64)
            for cbase, (e, toks, wts) in parts:
                if len(toks) == 0:
                    continue
                own = owner[toks]
                pos = np.zeros(len(toks), np.int64)
                for o in range(NCORES):
                    m = own == o
                    pos[m] = next_pos[o] + np.arange(m.sum())
                    next_pos[o] += m.sum()
                jr = np.arange(len(toks)) % 128
                jc = np.arange(len(toks)) // 128
                grids_sc[c][jr, cbase + jc] = own * p0 + pos
                grids_cw[c][jr, cbase + jc] = wts
                abs_rows = h * rows_h + c * p0 + pos
                sel = (topk_idx[toks] == e)
                k_of = sel[:, 1].astype(np.int64)
                slot_rows[toks, k_of] = abs_rows

    in_maps = []
    for c in range(NCORES):
        (_, h0_toks, _), (_, h1_toks, _), (be, b_toks, _) = core_groups[c]

        g0 = np.ascontiguousarray(
            slot_rows[c * TO:(c + 1) * TO, 0].astype(np.int32)
            .reshape(TO // 128, 128).T)
        g1 = np.ascontiguousarray(
            slot_rows[c * TO:(c + 1) * TO, 1].astype(np.int32)
            .reshape(TO // 128, 128).T)

        in_maps.append({
            "xg0": _xT_blocks(xr[h0_toks], hR),
            "xg1": _xT_blocks(xr[h1_toks], hR),
            "xgb": _xT_blocks(xr[b_toks], c_b),
            "xs": _xT_blocks(xr[c * TO:(c + 1) * TO], TO),
            "w1o": _tile_w1(w1r[c]), "w2o": _tile_w2(w2r[c]),
            "w1b": _tile_w1(w1r[be]), "w2b": _tile_w2(w2r[be]),
            "sw1": sw1_t, "sw2": sw2_t,
            "b1o": _col_bias(b1r[c]), "b1b": _col_bias(b1r[be]),
            "sb1": sb1_t,
            "b2o": np.ascontiguousarray(np.broadcast_to(b2r[c], (128, D))),
            "b2b": np.ascontiguousarray(np.broadcast_to(b2r[be], (128, D))),
            "sb2": np.ascontiguousarray(np.broadcast_to(
                np.asarray(shared_b2, np.float32)[0], (128, D))),
            "cw": grids_cw[c], "scat": grids_sc[c], "g0i": g0, "g1i": g1,
        })

    res = run_bass_kernel_spmd(nc, in_maps, list(range(NCORES)))
    return np.concatenate([res.results[c]["out"] for c in range(NCORES)],
                          axis=0).astype(np.float32)


# revision 22
# speedup vs baseline: 1.0279x; 1.0036x over previous
"""MoE (top-2 of 8 experts + 1 shared expert, SwiGLU FFN) on 8 TRN2 NeuronCores.

Strategy (expert-parallel + load-balanced overflow, per the sharding hint):
  - Host computes the (tiny) gate: softmax top-2 over E=8 for T=8192 tokens,
    and from it the dispatch. >99.9% of FLOPs (the FFNs) run on device.
  - Load balancing: expert loads vary (~1932..2182 for T=8192); SPMD padding
    to the max would cost ~5%. Instead each core runs its own expert's first
    R tokens (R ~ 1980, two half-passes) plus a small "borrowed" group
    (cap cB ~ 104) of overflow tokens from some hot expert, with that
    expert's weights supplied per-core. (R, cB) solve
    min R+cB s.t. sum_e ceil((c_e-R)+ / cB) <= 8, bringing per-core routed
    work from max_e c_e (~2184) down to ~mean+2% (~2084).
  - The borrowed group rides INSIDE half-pass 1 as a second token group:
    its weight stream (w1b/w2b, 24MB) amortizes over the whole half-pass.
    A standalone borrowed pass would need >500GB/s of weight bandwidth.
  - Each half-pass scales rows by the gate weight and scatters rows into an
    AllToAll dispatch buffer laid out by destination core; each half's
    AllToAll fires when ready, overlapping remaining compute. Each core also
    runs the shared expert on its own T/8 token slice.
  - Combine on device: out[t] = shared(t) + contrib0(t) + contrib1(t), where
    contribs are indirect-gathered from the a2a output segments (host
    precomputes absolute rows; placement is fully host-controlled).

Compute dtype is fp16 (PSUM accumulation fp32). The PE clock is power-capped
at ~1.95 GHz sustained (GPIO throttle k=13/16), so the kernel is tuned to
keep the MM stream dense: all DMAs are large partition-major transfers
(weights are host-tiled so each f-chunk / w2-slab is a single DMA), biases
are folded into the PSUM->SBUF drain, the per-row gate scale runs on the
scalar engine, and the finalize/combine work is interleaved per-tile with
the last stage-2 slab so nothing serializes after the final matmul. At
phase start, x chunks >=1 and the slab-0 w2 load are emitted behind the
first weight tiles so the first matmuls aren't queued behind bulk DMAs.
"""
import contextlib

import numpy as np

import concourse.bass as bass
import concourse.tile as tile
from concourse import bacc, mybir
from concourse.bass_utils import run_bass_kernel_spmd

# problem shape (hardcoded per contract)
T = 8192
D = 1024
F = 4096
E = 8
TOPK = 2
NCORES = 8
TO = T // NCORES          # tokens owned per core

F32 = mybir.dt.float32
F16 = mybir.dt.float16
I32 = mybir.dt.int32

MF = 2 * F // 128 // 2    # 32 f-chunks (a-half; b-half is mp+MF)
N_SLAB = 4
PER_SLAB = MF // N_SLAB   # 8 f-chunks per slab
KD = D // 128             # 8 contraction chunks for stage 1

_nc_cache: dict[tuple, object] = {}


def _chunk_slices(c_len):
    """Moving-dim chunks of <=512, each >=256 so LDWEIGHTS stays hidden.

    Largest chunk first: it gates the first matmul of a pass, and the
    extra matmul runway it provides covers the smaller chunks' transfers.
    """
    out = []
    rem = c_len
    while rem > 0:
        if rem <= 512:
            w = rem
        elif rem >= 768:
            w = 512
        else:
            w = rem - 256
        out.append(w)
        rem -= w
    out.sort(reverse=True)
    widths = []
    pos = 0
    for w in out:
        widths.append((pos, w))
        pos += w
    return widths


def _n_tiles(c_len):
    return -(-c_len // 128)


class _Group:
    """One token group within an FFN pass: its own x, weights, biases,
    output tiles and finalize callback."""

    def __init__(self, x_src, c_len, w1d, w2d, b1t, b2t, y_tiles, on_tile_d,
                 w1_eng, w2_engs, w1_pre=None, x_pre=None):
        self.x_src = x_src
        self.c_len = c_len
        self.w1d = w1d
        self.w2d = w2d
        self.b1t = b1t
        self.b2t = b2t
        self.y_tiles = y_tiles
        self.on_tile_d = on_tile_d
        self.w1_eng = w1_eng
        self.w2_engs = w2_engs
        self.w1_pre = w1_pre or {}   # {f-chunk: preloaded w1 tile}
        self.x_pre = x_pre           # preloaded [(xa, xb)] per chunk
        self.chunks = _chunk_slices(c_len)
        self.xk = []
        self.g_tiles = []
        self.w2t = None


def _ffn_phase(nc, sbuf, psA, psB, groups, x_eng, pre_loads=None,
               mid_loads=None):
    """Emit one SwiGLU FFN pass over one or more token groups.

    Per group: x_src is per-chunk DRAM APs [128, KD, 512]; w1d is DRAM
    [MF, 128, 2, KD, 128] fp16 (host-tiled; [...,0,...]=a, 1=b); w2d is
    DRAM [N_SLAB, 128, PER_SLAB, D] fp16; b1t SBUF [128, 2*MF] f32 (col m =
    bias for a-chunk m / b-chunk m-MF); b2t SBUF [128, D] f32 added once
    into y at slab 0; y_tiles receive FFN output + bias2;
    on_tile_d(q, t, rows, d, ds, last) runs after each stage-2 add.

    Every x chunk / k-half and each w2 half lives in its own tile so the
    DMAs carry no false whole-tile dependencies and transfer concurrently.
    Only the first chunk's x is loaded ahead of the first weight tile; the
    rest are deferred so the first matmul isn't queued behind bulk DMAs.
    """
    KH = KD // 2
    ei = 0
    deferred_x = []
    for gi, g in enumerate(groups):
        if g.x_pre is not None:
            g.xk = g.x_pre
            ei += len(g.chunks)
            continue
        g.xk = []
        for ci, (cs, cw) in enumerate(g.chunks):
            xa = sbuf.tile([128, KH, 512], F16, tag=f"x{gi}c{ci}a",
                           name=f"x{gi}c{ci}a", bufs=2 if gi == 0 else 1)
            xb = sbuf.tile([128, KH, 512], F16, tag=f"x{gi}c{ci}b",
                           name=f"x{gi}c{ci}b", bufs=2 if gi == 0 else 1)
            e0, e1 = x_eng[ei % len(x_eng)]
            ei += 1
            # the sync queue carries the weight stream: defer its x halves
            # so the first weight tiles aren't queued behind bulk x DMAs
            if gi == 0 and ci == 0:
                e0.dma_start(out=xa[:], in_=g.x_src[ci][:, 0:KH])
                e1.dma_start(out=xb[:], in_=g.x_src[ci][:, KH:KD])
            else:
                if e0 is not nc.sync:
                    e0.dma_start(out=xa[:], in_=g.x_src[ci][:, 0:KH])
                else:
                    deferred_x.append((e0, xa, g.x_src[ci], 0))
                if e1 is not nc.sync:
                    e1.dma_start(out=xb[:], in_=g.x_src[ci][:, KH:KD])
                else:
                    deferred_x.append((e1, xb, g.x_src[ci], 1))
            g.xk.append((xa, xb))
    if pre_loads is not None:
        pre_loads()

    for q in range(N_SLAB):
        for g in groups:
            g.g_tiles = []
        for fi in range(PER_SLAB):
            mp = q * PER_SLAB + fi
            if q == 1 and fi == 0 and mid_loads is not None:
                mid_loads()
            for gi, g in enumerate(groups):
                w1t = g.w1_pre.pop(mp, None)
                if w1t is None:
                    w1t = sbuf.tile([128, 2, KD, 128], F16, tag=f"w1t{gi}",
                                    name=f"w1t{gi}", bufs=8 if gi == 0 else 4)
                    g.w1_eng.dma_start(out=w1t[:], in_=g.w1d[mp])
                if q == 0 and fi == 0 and gi == 0:
                    # sync-side x halves of chunks >=1 ride behind the
                    # first weight tile in queue order (still ahead of
                    # their first consumers, emitted below)
                    for (e0, xt, src, hf) in deferred_x:
                        e0.dma_start(out=xt[:],
                                     in_=src[:, hf * KH:(hf + 1) * KH])
                g_t = sbuf.tile([128, g.c_len], F16, tag=f"g{gi}_{fi}",
                                name=f"g{gi}_{fi}", bufs=1)
                for ci, (cs, cw) in enumerate(g.chunks):
                    ps_a = psA.tile([128, 512], F32, space="PSUM", tag="ps_a",
                                    name="ps_a", bufs=3)
                    ps_b = psA.tile([128, 512], F32, space="PSUM", tag="ps_b",
                                    name="ps_b", bufs=3)
                    for k in range(KD):
                        rhs = g.xk[ci][k // KH][:, k % KH, :cw]
                        nc.tensor.matmul(out=ps_a[:, :cw],
                                         lhsT=w1t[:, 0, k, :], rhs=rhs,
                                         start=(k == 0), stop=(k == KD - 1))
                    for k in range(KD):
                        rhs = g.xk[ci][k // KH][:, k % KH, :cw]
                        nc.tensor.matmul(out=ps_b[:, :cw],
                                         lhsT=w1t[:, 1, k, :], rhs=rhs,
                                         start=(k == 0), stop=(k == KD - 1))
                    t_a = sbuf.tile([128, 512], F16, tag="t_a", name="t_a",
                                    bufs=6)
                    t_b = sbuf.tile([128, 512], F16, tag="t_b", name="t_b",
                                    bufs=6)
                    nc.scalar.activation(t_a[:, :cw], ps_a[:, :cw],
                                         mybir.ActivationFunctionType.Silu,
                                         bias=g.b1t[:, mp:mp + 1])
                    nc.scalar.activation(
                        t_b[:, :cw], ps_b[:, :cw],
                        mybir.ActivationFunctionType.Identity,
                        bias=g.b1t[:, mp + MF:mp + MF + 1])
                    nc.vector.tensor_mul(g_t[:, cs:cs + cw], t_a[:, :cw],
                                         t_b[:, :cw])
                g.g_tiles.append(g_t)
            # stage-2 weights for this slab (streamed during stage-1).
            # Slab 0's load is emitted mid-pass so it doesn't sit in front
            # of the first weight tiles in the queue at phase start.
            if fi == (5 if q == 0 else 0):
                for gi, g in enumerate(groups):
                    w2ta = sbuf.tile([128, PER_SLAB // 2, D], F16,
                                     tag=f"w2{gi}a", name=f"w2{gi}a", bufs=1)
                    w2tb = sbuf.tile([128, PER_SLAB // 2, D], F16,
                                     tag=f"w2{gi}b", name=f"w2{gi}b", bufs=1)
                    g.w2_engs[0].dma_start(out=w2ta[:],
                                           in_=g.w2d[q][:, 0:PER_SLAB // 2])
                    g.w2_engs[1].dma_start(
                        out=w2tb[:], in_=g.w2d[q][:, PER_SLAB // 2:PER_SLAB])
                    g.w2t = (w2ta, w2tb)
        # stage-2 partial: y (+)= g_slab.T @ w2_slab
        for g in groups:
            n_t = _n_tiles(g.c_len)
            w2ta, w2tb = g.w2t
            for t in range(n_t):
                rows = min(128, g.c_len - t * 128)
                ts = slice(t * 128, t * 128 + rows)
                for d in range(D // 512):
                    ds = slice(d * 512, (d + 1) * 512)
                    ps_y = psB.tile([128, 512], F32, space="PSUM",
                                    tag="ps_y", name="ps_y", bufs=2)
                    for fi in range(PER_SLAB):
                        w2s = w2ta if fi < PER_SLAB // 2 else w2tb
                        nc.tensor.matmul(
                            out=ps_y[:rows, :], lhsT=g.g_tiles[fi][:, ts],
                            rhs=w2s[:, fi % (PER_SLAB // 2), ds],
                            start=(fi == 0), stop=(fi == PER_SLAB - 1))
                    yt = g.y_tiles[t]
                    if q == 0:
                        nc.vector.tensor_add(yt[:rows, ds], ps_y[:rows, :],
                                             g.b2t[:rows, ds])
                    else:
                        nc.vector.tensor_add(yt[:rows, ds], yt[:rows, ds],
                                             ps_y[:rows, :])
                    g.on_tile_d(q, t, rows, d, ds, d == D // 512 - 1)


def _build(c_r, c_b, p0):
    """SPMD program: half-pass 0 (own expert, c_r/2 tokens), half-pass 1
    (own c_r/2 + borrowed c_b as a second group), shared pass. Each half
    scatters into its own AllToAll buffer; the shared pass gathers +
    combines + stores."""
    key = (c_r, c_b, p0)
    if key in _nc_cache:
        return _nc_cache[key]

    nc = bacc.Bacc("TRN2", target_bir_lowering=False, debug=False,
                   num_devices=NCORES)

    def din(name, shape, dt):
        return nc.dram_tensor(name, shape, dt, kind="ExternalInput").ap()

    hR = c_r // 2
    n_thR = _n_tiles(hR)                               # y tiles per R half
    n_thB = _n_tiles(c_b)
    n_to = TO // 128
    G = 2 * n_thR + n_thB                              # cw/scat grid columns
    rows_h = NCORES * p0                               # rows per half buffer

    n_chR = len(_chunk_slices(hR))
    n_chB = len(_chunk_slices(c_b))
    n_cs = len(_chunk_slices(TO))
    # gathered/owned tokens^T, one contiguous 512-wide block per chunk
    xg0 = din("xg0", [n_chR, 128, KD, 512], F16)
    xg1 = din("xg1", [n_chR, 128, KD, 512], F16)
    xgb = din("xgb", [n_chB, 128, KD, 512], F16)
    xs = din("xs", [n_cs, 128, KD, 512], F16)
    w1o = din("w1o", [MF, 128, 2, KD, 128], F16)
    w2o = din("w2o", [N_SLAB, 128, PER_SLAB, D], F16)
    w1b = din("w1b", [MF, 128, 2, KD, 128], F16)
    w2b = din("w2b", [N_SLAB, 128, PER_SLAB, D], F16)
    sw1 = din("sw1", [MF, 128, 2, KD, 128], F16)
    sw2 = din("sw2", [N_SLAB, 128, PER_SLAB, D], F16)
    b1o = din("b1o", [128, 2 * MF], F32)
    b1b = din("b1b", [128, 2 * MF], F32)
    sb1 = din("sb1", [128, 2 * MF], F32)
    b2o = din("b2o", [128, D], F32)
    b2b = din("b2b", [128, D], F32)
    sb2 = din("sb2", [128, D], F32)
    cwd = din("cw", [128, G], F32)                     # combine wt per col
    scat = din("scat", [128, G], I32)                  # row in half's a2a_in
    g0i = din("g0i", [128, n_to], I32)                 # abs row in a2a_out
    g1i = din("g1i", [128, n_to], I32)
    out = nc.dram_tensor("out", [TO, D], F16, kind="ExternalOutput").ap()

    with tile.TileContext(nc) as tc:
        with contextlib.ExitStack() as ctx:
            sbuf = ctx.enter_context(tc.tile_pool(name="sbuf", bufs=1))
            psA = ctx.enter_context(tc.tile_pool(name="psA", bufs=3,
                                                 space="PSUM"))
            psB = ctx.enter_context(tc.tile_pool(name="psB", bufs=2,
                                                 space="PSUM"))
            dpool = ctx.enter_context(tc.tile_pool(name="dram", bufs=1,
                                                   space="DRAM"))

            a2a_in0 = dpool.tile([rows_h, D], F16)
            a2a_in1 = dpool.tile([rows_h, D], F16)
            a2a_out = dpool.tile([2 * rows_h, D], F16)
            a2a_ins = [a2a_in0, a2a_in1]

            # PE warmup: trip the HAM activity window during the input
            # DMAs so the first real matmuls run at full clock. Fed from a
            # memset tile so no DMA gates it.
            wu = sbuf.tile([128, 512], F16, tag="wu", name="wu", bufs=1)
            nc.vector.memset(wu[:], 1.0)
            for _ in range(34):
                ps_w = psA.tile([128, 512], F32, space="PSUM", tag="ps_a",
                                name="ps_a", bufs=3)
                nc.tensor.matmul(out=ps_w[:1, :], lhsT=wu[:, :1],
                                 rhs=wu[:], start=True, stop=True)

            # biases + index grids (resident; off the sync queue, which is
            # reserved for the bulk x/w stream)
            b1ot = sbuf.tile([128, 2 * MF], F32, tag="b1ot", name="b1ot",
                             bufs=1)
            b1bt = sbuf.tile([128, 2 * MF], F32, tag="b1bt", name="b1bt",
                             bufs=1)
            sb1t = sbuf.tile([128, 2 * MF], F32, tag="sb1t", name="sb1t",
                             bufs=1)
            cwt = sbuf.tile([128, G], F32, tag="cwt", name="cwt", bufs=1)
            sct = sbuf.tile([128, G], I32, tag="sct", name="sct", bufs=1)
            b2ot = sbuf.tile([128, D], F32, tag="b2ot", name="b2ot", bufs=1)
            b2bt = sbuf.tile([128, D], F32, tag="b2bt", name="b2bt", bufs=1)
            sb2t = sbuf.tile([128, D], F32, tag="sb2t", name="sb2t", bufs=1)
            nc.scalar.dma_start(out=b1ot[:], in_=b1o[:])
            nc.gpsimd.dma_start(out=b1bt[:], in_=b1b[:])
            nc.gpsimd.dma_start(out=cwt[:], in_=cwd[:])
            nc.gpsimd.dma_start(out=sct[:], in_=scat[:])
            nc.gpsimd.dma_start(out=sb1t[:], in_=sb1[:])

            def mk_fin(cbase, y_tiles, a2a_in, yh_box):
                def fin_tile(q, t, rows, d, ds, last):
                    # *combine weight (scalar engine), scatter to a2a buf
                    if q != N_SLAB - 1:
                        return
                    col = cbase + t
                    if d == 0:
                        yh_box[0] = sbuf.tile([128, D], F16, tag="yh",
                                              name="yh", bufs=2)
                    yh = yh_box[0]
                    nc.scalar.activation(yh[:rows, ds], y_tiles[t][:rows, ds],
                                         mybir.ActivationFunctionType.Copy,
                                         scale=cwt[:rows, col:col + 1])
                    if last:
                        nc.gpsimd.indirect_dma_start(
                            out=a2a_in[:],
                            out_offset=bass.IndirectOffsetOnAxis(
                                ap=sct[:rows, col:col + 1], axis=0),
                            in_=yh[:rows, :],
                            in_offset=None,
                            bounds_check=rows_h - 1,
                            oob_is_err=False,
                        )
                return fin_tile

            # ---------------- routed expert (2 half-passes over tokens) ----
            x_eng3 = [(nc.sync, nc.scalar), (nc.gpsimd, nc.sync),
                      (nc.scalar, nc.gpsimd)]

            # Borrowed-stream head start: the scheduler gates each w1 tile
            # DMA on PE progress (slot reuse), so queue lookahead is capped
            # at `bufs` and evaporates at the half-0/half-1 boundary right
            # when the first AllToAll saturates DMA. The borrowed tags are
            # virgin during half 0, so their first tiles + x prefetch
            # ungated, emitted mid-half-0 (clear of the startup window).
            w1b_pre = {}
            xgb_pre = []

            def h0_mid():
                for m in range(4):
                    w1t = sbuf.tile([128, 2, KD, 128], F16, tag="w1t1",
                                    name="w1t1", bufs=4)
                    nc.gpsimd.dma_start(out=w1t[:], in_=w1b[m])
                    w1b_pre[m] = w1t
                KH = KD // 2
                for ci in range(n_chB):
                    xa = sbuf.tile([128, KH, 512], F16, tag=f"x1c{ci}a",
                                   name=f"x1c{ci}a", bufs=1)
                    xb = sbuf.tile([128, KH, 512], F16, tag=f"x1c{ci}b",
                                   name=f"x1c{ci}b", bufs=1)
                    nc.gpsimd.dma_start(out=xa[:], in_=xgb[ci][:, 0:KH])
                    nc.gpsimd.dma_start(out=xb[:], in_=xgb[ci][:, KH:KD])
                    xgb_pre.append((xa, xb))

            for h in range(2):
                y_tiles = [sbuf.tile([128, D], F16, tag=f"ya{t}",
                                     name=f"ya{t}", bufs=1)
                           for t in range(n_thR)]
                own = _Group([xg0, xg1][h], hR, w1o, w2o, b1ot, b2ot,
                             y_tiles,
                             mk_fin(h * n_thR, y_tiles, a2a_ins[h], [None]),
                             nc.sync, (nc.sync, nc.scalar))
                grps = [own]
                if h == 1:
                    yb_tiles = [sbuf.tile([128, D], F16, tag=f"yb{t}",
                                          name=f"yb{t}", bufs=1)
                                for t in range(n_thB)]
                    grps.append(_Group(
                        xgb, c_b, w1b, w2b, b1bt, b2bt, yb_tiles,
                        mk_fin(2 * n_thR, yb_tiles, a2a_ins[1], [None]),
                        nc.sync, (nc.gpsimd, nc.gpsimd),
                        w1_pre=w1b_pre, x_pre=xgb_pre))
                pre = ((lambda: nc.gpsimd.dma_start(out=b2ot[:], in_=b2o[:]))
                       if h == 0 else
                       (lambda: nc.gpsimd.dma_start(out=b2bt[:], in_=b2b[:])))
                _ffn_phase(nc, sbuf, psA, psB, grps, x_eng3, pre_loads=pre,
                           mid_loads=h0_mid if h == 0 else None)
                # dispatch this half back to the combiner cores
                nc.gpsimd.collective_compute(
                    "AllToAll",
                    mybir.AluOpType.bypass,
                    replica_groups=[list(range(NCORES))],
                    ins=[a2a_ins[h][:].opt()],
                    outs=[a2a_out[h * rows_h:(h + 1) * rows_h, :].opt()],
                )

            # ---------------- shared expert on owned tokens (overlaps) ----
            i0 = sbuf.tile([128, n_to], I32, tag="i0", name="i0", bufs=1)
            i1 = sbuf.tile([128, n_to], I32, tag="i1", name="i1", bufs=1)
            nc.scalar.dma_start(out=i0[:], in_=g0i[:])
            nc.scalar.dma_start(out=i1[:], in_=g1i[:])
            # gathers of routed contributions are emitted via pre_loads so
            # the gpsimd queue serves the shared phase's x pieces first;
            # they overlap shared compute (waiting on both AllToAlls via
            # the a2a_out dependency)
            r0s, r1s = [], []

            def shared_pre():
                nc.gpsimd.dma_start(out=sb2t[:], in_=sb2[:])
                for t in range(n_to):
                    r0 = sbuf.tile([128, D], F16, tag="r0", name="r0", bufs=4)
                    r1 = sbuf.tile([128, D], F16, tag="r1", name="r1", bufs=4)
                    nc.gpsimd.indirect_dma_start(
                        out=r0[:], out_offset=None, in_=a2a_out[:],
                        in_offset=bass.IndirectOffsetOnAxis(
                            ap=i0[:, t:t + 1], axis=0))
                    nc.gpsimd.indirect_dma_start(
                        out=r1[:], out_offset=None, in_=a2a_out[:],
                        in_offset=bass.IndirectOffsetOnAxis(
                            ap=i1[:, t:t + 1], axis=0))
                    r0s.append(r0)
                    r1s.append(r1)

            ys_tiles = [sbuf.tile([128, D], F16, tag=f"ya{t}", name=f"ya{t}",
                                  bufs=1)
                        for t in range(n_to)]

            def comb_tile(q, t, rows, d, ds, last):
                # routed contributions fold in one slab early (commutative);
                # after the final slab only a half-tile store trails the
                # last matmul
                if q == N_SLAB - 2:
                    yt = ys_tiles[t]
                    nc.vector.tensor_add(yt[:, ds], yt[:, ds], r0s[t][:, ds])
                    nc.vector.tensor_add(yt[:, ds], yt[:, ds], r1s[t][:, ds])
                elif q == N_SLAB - 1:
                    tr = slice(t * 128, (t + 1) * 128)
                    if t == n_to - 1 and last:
                        h0 = slice(ds.start, ds.start + 256)
                        h1 = slice(ds.start + 256, ds.stop)
                        nc.sync.dma_start(out=out[tr, h0],
                                          in_=ys_tiles[t][:, h0])
                        nc.scalar.dma_start(out=out[tr, h1],
                                            in_=ys_tiles[t][:, h1])
                    else:
                        eng = nc.sync if (t + d) % 2 == 0 else nc.scalar
                        eng.dma_start(out=out[tr, ds],
                                      in_=ys_tiles[t][:, ds])

            shared_g = _Group(xs, TO, sw1, sw2, sb1t, sb2t, ys_tiles,
                              comb_tile, nc.sync, (nc.sync, nc.scalar))
            _ffn_phase(nc, sbuf, psA, psB, [shared_g], x_eng3,
                       pre_loads=shared_pre)

    nc.compile()
    _nc_cache[key] = nc
    return nc


def _route(x, gate_w, gate_b):
    """Host gate: softmax top-2 (float64 for stable ordering)."""
    logits = (x.astype(np.float64) @ gate_w.astype(np.float64)
              + gate_b.astype(np.float64))
    m = logits.max(axis=-1, keepdims=True)
    p = np.exp(logits - m)
    p /= p.sum(axis=-1, keepdims=True)
    order = np.argsort(-p, axis=-1)
    idx = order[:, :TOPK]                      # [T, 2]
    wts = np.take_along_axis(p, idx, axis=-1)  # [T, 2]
    return idx, wts.astype(np.float32)


def _solve_caps(loads):
    """min R+B s.t. sum_e ceil((c_e-R)+ / B) <= NCORES, caps mult of 4."""
    best = None
    for R in range(1024, max(loads) + 4, 4):
        ovf = [c - R for c in loads if c > R]
        if not ovf:
            if best is None or R + 8 < best[0] + best[1]:
                best = (R, 8)
            break
        B = 4
        while B <= R:
            if sum(-(-o // B) for o in ovf) <= NCORES:
                break
            B += 4
        else:
            continue
        if best is None or R + B < best[0] + best[1]:
            best = (R, B)
    return best


def _tile_w1(w):      # [D, 2F] -> [MF, 128, 2, KD, 128]
    v = np.asarray(w, np.float16).reshape(KD, 128, 2 * MF, 128)
    s = np.stack([v[:, :, :MF, :], v[:, :, MF:, :]], axis=0)  # [j,k,p,m,c]
    return np.ascontiguousarray(s.transpose(3, 2, 0, 1, 4))


def _tile_w2(w):      # [F, D] -> [N_SLAB, 128, PER_SLAB, D]
    v = np.asarray(w, np.float16).reshape(N_SLAB, PER_SLAB, 128, D)
    return np.ascontiguousarray(v.transpose(0, 2, 1, 3))


def _col_bias(b):     # [2F] -> [128, 2*MF]
    return np.ascontiguousarray(
        np.asarray(b, np.float32).reshape(2 * MF, 128).T)


def _xT_blocks(xr, c_cap):
    """[C, D] fp16 -> per-chunk contiguous blocks [n_ch, 128, KD, 512]."""
    ct = np.zeros((128, KD, c_cap), np.float16)
    n = len(xr)
    if n:
        ct[:, :, :n] = xr.T.reshape(KD, 128, n).transpose(1, 0, 2)
    chunks = _chunk_slices(c_cap)
    blk = np.zeros((len(chunks), 128, KD, 512), np.float16)
    for ci, (cs, cw) in enumerate(chunks):
        blk[ci, :, :, :cw] = ct[:, :, cs:cs + cw]
    return blk


def kernel(x, gate_w, gate_b, shared_w1, shared_b1, shared_w2, shared_b2,
           routed_w1, routed_b1, routed_w2, routed_b2):
    x = np.asarray(x, dtype=np.float32)
    topk_idx, topk_w = _route(x, np.asarray(gate_w), np.asarray(gate_b))

    owner = np.arange(T) // TO                 # owning core per token

    # per-expert dispatch lists, ordered by (owner, token)
    tok_lists, wt_lists = [], []
    for e in range(E):
        sel = (topk_idx == e)                  # [T, 2]
        tsel = np.nonzero(sel.any(axis=1))[0]  # ascending => owner-sorted
        k_of = sel[tsel, 1].astype(np.int64)   # slot (experts distinct)
        w_of = topk_w[tsel, :][np.arange(len(tsel)), k_of]
        tok_lists.append(tsel)
        wt_lists.append(w_of)

    loads = [len(t) for t in tok_lists]
    c_r, c_b = _solve_caps(loads)
    hR = c_r // 2
    n_thR = _n_tiles(hR)
    n_thB = _n_tiles(c_b)
    G = 2 * n_thR + n_thB

    # borrowed pieces: overflow of hot experts, one piece per core
    empty_t = np.zeros(0, np.int64)
    empty_w = np.zeros(0, np.float32)
    pieces = []
    for e in range(E):
        for s in range(min(loads[e], c_r), loads[e], c_b):
            pieces.append((e, tok_lists[e][s:s + c_b],
                           wt_lists[e][s:s + c_b]))
    assert len(pieces) <= NCORES, (loads, c_r, c_b)
    # per-core groups: [half0 own, half1 own, half1 borrowed]
    core_groups = []
    for c in range(NCORES):
        bw = pieces[c] if c < len(pieces) else (c, empty_t, empty_w)
        own_t, own_w = tok_lists[c], wt_lists[c]
        core_groups.append([(c, own_t[:hR], own_w[:hR]),
                            (c, own_t[hR:c_r], own_w[hR:c_r]), bw])

    # a2a row cap: per (src core, half, dest core) group sizes; half 1
    # packs own + borrowed tokens jointly per dest
    p0 = 1
    for c in range(NCORES):
        g0, g1, gb = core_groups[c]
        cnt0 = np.bincount(owner[g0[1]], minlength=NCORES)
        cnt1 = (np.bincount(owner[g1[1]], minlength=NCORES)
                + np.bincount(owner[gb[1]], minlength=NCORES))
        p0 = max(p0, int(cnt0.max()), int(cnt1.max()))
    p0 = -(-p0 // 8) * 8
    rows_h = NCORES * p0

    nc = _build(c_r, c_b, p0)

    # host-side layouts (fp16 compute dtype)
    w1r = np.asarray(routed_w1, np.float16)              # [E, D, 2F]
    w2r = np.asarray(routed_w2, np.float16)              # [E, F, D]
    b1r = np.asarray(routed_b1)                          # [E, 2F]
    b2r = np.asarray(routed_b2, np.float32)              # [E, D]
    xr = x.astype(np.float16)                            # [T, D]

    sw1_t = _tile_w1(np.asarray(shared_w1, np.float16)[0])
    sw2_t = _tile_w2(np.asarray(shared_w2, np.float16)[0])
    sb1_t = _col_bias(np.asarray(shared_b1)[0])

    # absolute a2a_out row for each (token, slot); combine-wt/scatter grids.
    # Grid columns: half0 own tiles [0, n_thR), half1 own [n_thR, 2*n_thR),
    # half1 borrowed [2*n_thR, G). Scatter row (within the half's a2a_in)
    # = dest*p0 + pos, pos running jointly over (own, borrowed) per dest.
    slot_rows = np.zeros((T, TOPK), np.int64)
    grids_cw = [np.zeros((128, G), np.float32) for _ in range(NCORES)]
    grids_sc = [np.full((128, G), 2**31 - 1, np.int32) for _ in range(NCORES)]
    for c in range(NCORES):
        for h in range(2):
            if h == 0:
                parts = [(0, core_groups[c][0])]
            else:
                parts = [(n_thR, core_groups[c][1]),
                         (2 * n_thR, core_groups[c][2])]
            next_pos = np.zeros(NCORES, np.int64)
            for cbase, (e, toks, wts) in parts:
                if len(toks) == 0:
                    continue
                own = owner[toks]
                pos = np.zeros(len(toks), np.int64)
                for o in range(NCORES):
                    m = own == o
                    pos[m] = next_pos[o] + np.arange(m.sum())
                    next_pos[o] += m.sum()
                jr = np.arange(len(toks)) % 128
                jc = np.arange(len(toks)) // 128
                grids_sc[c][jr, cbase + jc] = own * p0 + pos
                grids_cw[c][jr, cbase + jc] = wts
                abs_rows = h * rows_h + c * p0 + pos
                sel = (topk_idx[toks] == e)
                k_of = sel[:, 1].astype(np.int64)
                slot_rows[toks, k_of] = abs_rows

    in_maps = []
    for c in range(NCORES):
        (_, h0_toks, _), (_, h1_toks, _), (be, b_toks, _) = core_groups[c]

        g0 = np.ascontiguousarray(
            slot_rows[c * TO:(c + 1) * TO, 0].astype(np.int32)
            .reshape(TO // 128, 128).T)
        g1 = np.ascontiguousarray(
            slot_rows[c * TO:(c + 1) * TO, 1].astype(np.int32)
            .reshape(TO // 128, 128).T)

        in_maps.append({
            "xg0": _xT_blocks(xr[h0_toks], hR),
            "xg1": _xT_blocks(xr[h1_toks], hR),
            "xgb": _xT_blocks(xr[b_toks], c_b),
            "xs": _xT_blocks(xr[c * TO:(c + 1) * TO], TO),
            "w1o": _tile_w1(w1r[c]), "w2o": _tile_w2(w2r[c]),
            "w1b": _tile_w1(w1r[be]), "w2b": _tile_w2(w2r[be]),
            "sw1": sw1_t, "sw2": sw2_t,
            "b1o": _col_bias(b1r[c]), "b1b": _col_bias(b1r[be]),
            "sb1": sb1_t,
            "b2o": np.ascontiguousarray(np.broadcast_to(b2r[c], (128, D))),
            "b2b": np.ascontiguousarray(np.broadcast_to(b2r[be], (128, D))),
            "sb2": np.ascontiguousarray(np.broadcast_to(
                np.asarray(shared_b2, np.float32)[0], (128, D))),
            "cw": grids_cw[c], "scat": grids_sc[c], "g0i": g0, "g1i": g1,
        })

    res = run_bass_kernel_spmd(nc, in_maps, list(range(NCORES)))
    return np.concatenate([res.results[c]["out"] for c in range(NCORES)],
                          axis=0).astype(np.float32)


# revision 25
# speedup vs baseline: 1.0458x; 1.0174x over previous
"""MoE (top-2 of 8 experts + 1 shared expert, SwiGLU FFN) on 8 TRN2 NeuronCores.

Strategy (expert-parallel + load-balanced overflow, per the sharding hint):
  - Host computes the (tiny) gate: softmax top-2 over E=8 for T=8192 tokens,
    and from it the dispatch. >99.9% of FLOPs (the FFNs) run on device.
  - Load balancing: expert loads vary (~1932..2182 for T=8192); SPMD padding
    to the max would cost ~5%. Instead each core runs its own expert's first
    R tokens (R ~ 1980, two half-passes) plus a small "borrowed" group
    (cap cB ~ 104) of overflow tokens from some hot expert, with that
    expert's weights supplied per-core. (R, cB) solve
    min R+cB s.t. sum_e ceil((c_e-R)+ / cB) <= 8, bringing per-core routed
    work from max_e c_e (~2184) down to ~mean+2% (~2084).
  - The borrowed group rides INSIDE half-pass 1 as a second token group:
    its weight stream (w1b/w2b, 24MB) amortizes over the whole half-pass.
    A standalone borrowed pass would need >500GB/s of weight bandwidth.
  - Each half-pass scales rows by the gate weight and scatters rows into an
    AllToAll dispatch buffer laid out by destination core; each half's
    AllToAll fires when ready, overlapping remaining compute. Each core also
    runs the shared expert on its own T/8 token slice.
  - Combine on device: out[t] = shared(t) + contrib0(t) + contrib1(t), where
    contribs are indirect-gathered from the a2a output segments (host
    precomputes absolute rows; placement is fully host-controlled).

Compute dtype is fp16 (PSUM accumulation fp32). The PE clock is power-capped
at ~1.95 GHz sustained (GPIO throttle k=13/16), so the kernel is tuned to
keep the MM stream dense: all DMAs are large partition-major transfers
(weights are host-tiled so each f-chunk / w2-slab is a single DMA), biases
are folded into the PSUM->SBUF drain, the per-row gate scale runs on the
scalar engine, and the finalize/combine work is interleaved per-tile with
the last stage-2 slab so nothing serializes after the final matmul. At
phase start, x chunks >=1 and the slab-0 w2 load are emitted behind the
first weight tiles so the first matmuls aren't queued behind bulk DMAs.
"""
import contextlib

import numpy as np

import concourse.bass as bass
import concourse.tile as tile
from concourse import bacc, mybir
from concourse.bass_utils import run_bass_kernel_spmd

# problem shape (hardcoded per contract)
T = 8192
D = 1024
F = 4096
E = 8
TOPK = 2
NCORES = 8
TO = T // NCORES          # tokens owned per core

F32 = mybir.dt.float32
F16 = mybir.dt.float16
I32 = mybir.dt.int32

MF = 2 * F // 128 // 2    # 32 f-chunks (a-half; b-half is mp+MF)
N_SLAB = 4
PER_SLAB = MF // N_SLAB   # 8 f-chunks per slab
KD = D // 128             # 8 contraction chunks for stage 1

_nc_cache: dict[tuple, object] = {}


def _chunk_slices(c_len):
    """Moving-dim chunks of <=512, each >=256 so LDWEIGHTS stays hidden.

    Largest chunk first: it gates the first matmul of a pass, and the
    extra matmul runway it provides covers the smaller chunks' transfers.
    """
    out = []
    rem = c_len
    while rem > 0:
        if rem <= 512:
            w = rem
        elif rem >= 768:
            w = 512
        else:
            w = rem - 256
        out.append(w)
        rem -= w
    out.sort(reverse=True)
    widths = []
    pos = 0
    for w in out:
        widths.append((pos, w))
        pos += w
    return widths


def _n_tiles(c_len):
    return -(-c_len // 128)


class _Group:
    """One token group within an FFN pass: its own x, weights, biases,
    output tiles and finalize callback."""

    def __init__(self, x_src, c_len, w1d, w2d, b1t, b2t, y_tiles, on_tile_d,
                 w1_eng, w2_engs, w1_pre=None, x_pre=None):
        self.x_src = x_src
        self.c_len = c_len
        self.w1d = w1d
        self.w2d = w2d
        self.b1t = b1t
        self.b2t = b2t
        self.y_tiles = y_tiles
        self.on_tile_d = on_tile_d
        self.w1_eng = w1_eng
        self.w2_engs = w2_engs
        self.w1_pre = w1_pre or {}   # {f-chunk: preloaded w1 tile}
        self.x_pre = x_pre           # preloaded [(xa, xb)] per chunk
        self.chunks = _chunk_slices(c_len)
        self.xk = []
        self.g_tiles = []
        self.w2t = None


def _ffn_phase(nc, sbuf, psA, psB, groups, x_eng, pre_loads=None,
               mid_loads=None):
    """Emit one SwiGLU FFN pass over one or more token groups.

    Per group: x_src is per-chunk DRAM APs [128, KD, 512]; w1d is DRAM
    [MF, 128, 2, KD, 128] fp16 (host-tiled; [...,0,...]=a, 1=b); w2d is
    DRAM [N_SLAB, 128, PER_SLAB, D] fp16; b1t SBUF [128, 2*MF] f32 (col m =
    bias for a-chunk m / b-chunk m-MF); b2t SBUF [128, D] f32 added once
    into y at slab 0; y_tiles receive FFN output + bias2;
    on_tile_d(q, t, rows, d, ds, last) runs after each stage-2 add.

    Every x chunk / k-half and each w2 half lives in its own tile so the
    DMAs carry no false whole-tile dependencies and transfer concurrently.
    Only the first chunk's x is loaded ahead of the first weight tile; the
    rest are deferred so the first matmul isn't queued behind bulk DMAs.
    """
    KH = KD // 2
    ei = 0
    deferred_x = []
    for gi, g in enumerate(groups):
        if g.x_pre is not None:
            g.xk = g.x_pre
            ei += len(g.chunks)
            continue
        g.xk = []
        for ci, (cs, cw) in enumerate(g.chunks):
            xa = sbuf.tile([128, KH, 512], F16, tag=f"x{gi}c{ci}a",
                           name=f"x{gi}c{ci}a", bufs=2 if gi == 0 else 1)
            xb = sbuf.tile([128, KH, 512], F16, tag=f"x{gi}c{ci}b",
                           name=f"x{gi}c{ci}b", bufs=2 if gi == 0 else 1)
            e0, e1 = x_eng[ei % len(x_eng)]
            ei += 1
            # the sync queue carries the weight stream: defer its x halves
            # so the first weight tiles aren't queued behind bulk x DMAs
            if gi == 0 and ci == 0:
                e0.dma_start(out=xa[:], in_=g.x_src[ci][:, 0:KH])
                e1.dma_start(out=xb[:], in_=g.x_src[ci][:, KH:KD])
            else:
                if e0 is not nc.sync:
                    e0.dma_start(out=xa[:], in_=g.x_src[ci][:, 0:KH])
                else:
                    deferred_x.append((e0, xa, g.x_src[ci], 0))
                if e1 is not nc.sync:
                    e1.dma_start(out=xb[:], in_=g.x_src[ci][:, KH:KD])
                else:
                    deferred_x.append((e1, xb, g.x_src[ci], 1))
            g.xk.append((xa, xb))
    if pre_loads is not None:
        pre_loads()

    for q in range(N_SLAB):
        for g in groups:
            g.g_tiles = []
        for fi in range(PER_SLAB):
            mp = q * PER_SLAB + fi
            if q == 1 and fi == 0 and mid_loads is not None:
                mid_loads()
            for gi, g in enumerate(groups):
                w1t = g.w1_pre.pop(mp, None)
                if w1t is None:
                    w1t = sbuf.tile([128, 2, KD, 128], F16, tag=f"w1t{gi}",
                                    name=f"w1t{gi}", bufs=8 if gi == 0 else 5)
                    g.w1_eng.dma_start(out=w1t[:], in_=g.w1d[mp])
                if q == 0 and fi == 0 and gi == 0:
                    # sync-side x halves of chunks >=1 ride behind the
                    # first weight tile in queue order (still ahead of
                    # their first consumers, emitted below)
                    for (e0, xt, src, hf) in deferred_x:
                        e0.dma_start(out=xt[:],
                                     in_=src[:, hf * KH:(hf + 1) * KH])
                g_t = sbuf.tile([128, g.c_len], F16, tag=f"g{gi}_{fi}",
                                name=f"g{gi}_{fi}", bufs=1)
                for ci, (cs, cw) in enumerate(g.chunks):
                    ps_a = psA.tile([128, 512], F32, space="PSUM", tag="ps_a",
                                    name="ps_a", bufs=3)
                    ps_b = psA.tile([128, 512], F32, space="PSUM", tag="ps_b",
                                    name="ps_b", bufs=3)
                    for k in range(KD):
                        rhs = g.xk[ci][k // KH][:, k % KH, :cw]
                        nc.tensor.matmul(out=ps_a[:, :cw],
                                         lhsT=w1t[:, 0, k, :], rhs=rhs,
                                         start=(k == 0), stop=(k == KD - 1))
                    for k in range(KD):
                        rhs = g.xk[ci][k // KH][:, k % KH, :cw]
                        nc.tensor.matmul(out=ps_b[:, :cw],
                                         lhsT=w1t[:, 1, k, :], rhs=rhs,
                                         start=(k == 0), stop=(k == KD - 1))
                    t_a = sbuf.tile([128, 512], F16, tag="t_a", name="t_a",
                                    bufs=6)
                    t_b = sbuf.tile([128, 512], F16, tag="t_b", name="t_b",
                                    bufs=6)
                    nc.scalar.activation(t_a[:, :cw], ps_a[:, :cw],
                                         mybir.ActivationFunctionType.Silu,
                                         bias=g.b1t[:, mp:mp + 1])
                    nc.scalar.activation(
                        t_b[:, :cw], ps_b[:, :cw],
                        mybir.ActivationFunctionType.Identity,
                        bias=g.b1t[:, mp + MF:mp + MF + 1])
                    nc.vector.tensor_mul(g_t[:, cs:cs + cw], t_a[:, :cw],
                                         t_b[:, :cw])
                g.g_tiles.append(g_t)
            # stage-2 weights for this slab (streamed during stage-1).
            # Slab 0's load is emitted mid-pass so it doesn't sit in front
            # of the first weight tiles in the queue at phase start.
            if fi == (5 if q == 0 else 0):
                for gi, g in enumerate(groups):
                    w2ta = sbuf.tile([128, PER_SLAB // 2, D], F16,
                                     tag=f"w2{gi}a", name=f"w2{gi}a", bufs=1)
                    w2tb = sbuf.tile([128, PER_SLAB // 2, D], F16,
                                     tag=f"w2{gi}b", name=f"w2{gi}b", bufs=1)
                    g.w2_engs[0].dma_start(out=w2ta[:],
                                           in_=g.w2d[q][:, 0:PER_SLAB // 2])
                    g.w2_engs[1].dma_start(
                        out=w2tb[:], in_=g.w2d[q][:, PER_SLAB // 2:PER_SLAB])
                    g.w2t = (w2ta, w2tb)
        # stage-2 partial: y (+)= g_slab.T @ w2_slab
        for g in groups:
            n_t = _n_tiles(g.c_len)
            w2ta, w2tb = g.w2t
            for t in range(n_t):
                rows = min(128, g.c_len - t * 128)
                ts = slice(t * 128, t * 128 + rows)
                for d in range(D // 512):
                    ds = slice(d * 512, (d + 1) * 512)
                    ps_y = psB.tile([128, 512], F32, space="PSUM",
                                    tag="ps_y", name="ps_y", bufs=2)
                    for fi in range(PER_SLAB):
                        w2s = w2ta if fi < PER_SLAB // 2 else w2tb
                        nc.tensor.matmul(
                            out=ps_y[:rows, :], lhsT=g.g_tiles[fi][:, ts],
                            rhs=w2s[:, fi % (PER_SLAB // 2), ds],
                            start=(fi == 0), stop=(fi == PER_SLAB - 1))
                    yt = g.y_tiles[t]
                    if q == 0:
                        nc.vector.tensor_add(yt[:rows, ds], ps_y[:rows, :],
                                             g.b2t[:rows, ds])
                    else:
                        nc.vector.tensor_add(yt[:rows, ds], yt[:rows, ds],
                                             ps_y[:rows, :])
                    g.on_tile_d(q, t, rows, d, ds, d == D // 512 - 1)


def _build(c_r, c_b, p0):
    """SPMD program: half-pass 0 (own expert, c_r/2 tokens), half-pass 1
    (own c_r/2 + borrowed c_b as a second group), shared pass. Each half
    scatters into its own AllToAll buffer; the shared pass gathers +
    combines + stores."""
    key = (c_r, c_b, p0)
    if key in _nc_cache:
        return _nc_cache[key]

    nc = bacc.Bacc("TRN2", target_bir_lowering=False, debug=False,
                   num_devices=NCORES)

    def din(name, shape, dt):
        return nc.dram_tensor(name, shape, dt, kind="ExternalInput").ap()

    hR = c_r // 2
    n_thR = _n_tiles(hR)                               # y tiles per R half
    n_thB = _n_tiles(c_b)
    n_to = TO // 128
    G = 2 * n_thR + n_thB                              # cw/scat grid columns
    rows_h = NCORES * p0                               # rows per half buffer

    n_chR = len(_chunk_slices(hR))
    n_chB = len(_chunk_slices(c_b))
    n_cs = len(_chunk_slices(TO))
    # gathered/owned tokens^T, one contiguous 512-wide block per chunk
    xg0 = din("xg0", [n_chR, 128, KD, 512], F16)
    xg1 = din("xg1", [n_chR, 128, KD, 512], F16)
    xgb = din("xgb", [n_chB, 128, KD, 512], F16)
    xs = din("xs", [n_cs, 128, KD, 512], F16)
    w1o = din("w1o", [MF, 128, 2, KD, 128], F16)
    w2o = din("w2o", [N_SLAB, 128, PER_SLAB, D], F16)
    w1b = din("w1b", [MF, 128, 2, KD, 128], F16)
    w2b = din("w2b", [N_SLAB, 128, PER_SLAB, D], F16)
    sw1 = din("sw1", [MF, 128, 2, KD, 128], F16)
    sw2 = din("sw2", [N_SLAB, 128, PER_SLAB, D], F16)
    b1o = din("b1o", [128, 2 * MF], F32)
    b1b = din("b1b", [128, 2 * MF], F32)
    sb1 = din("sb1", [128, 2 * MF], F32)
    b2o = din("b2o", [128, D], F32)
    b2b = din("b2b", [128, D], F32)
    sb2 = din("sb2", [128, D], F32)
    cwd = din("cw", [128, G], F32)                     # combine wt per col
    scat = din("scat", [128, G], I32)                  # row in half's a2a_in
    g0i = din("g0i", [128, n_to], I32)                 # abs row in a2a_out
    g1i = din("g1i", [128, n_to], I32)
    out = nc.dram_tensor("out", [TO, D], F16, kind="ExternalOutput").ap()

    with tile.TileContext(nc) as tc:
        with contextlib.ExitStack() as ctx:
            sbuf = ctx.enter_context(tc.tile_pool(name="sbuf", bufs=1))
            psA = ctx.enter_context(tc.tile_pool(name="psA", bufs=3,
                                                 space="PSUM"))
            psB = ctx.enter_context(tc.tile_pool(name="psB", bufs=2,
                                                 space="PSUM"))
            dpool = ctx.enter_context(tc.tile_pool(name="dram", bufs=1,
                                                   space="DRAM"))

            a2a_in0 = dpool.tile([rows_h, D], F16)
            a2a_in1 = dpool.tile([rows_h, D], F16)
            a2a_out = dpool.tile([2 * rows_h, D], F16)
            a2a_ins = [a2a_in0, a2a_in1]

            # PE warmup: trip the HAM activity window during the input
            # DMAs so the first real matmuls run at full clock. Fed from a
            # memset tile so no DMA gates it. Sized to end about when the
            # first x/w tiles land (~6us) -- overshoot delays real work.
            wu = sbuf.tile([128, 512], F16, tag="wu", name="wu", bufs=1)
            nc.vector.memset(wu[:], 1.0)
            for wi in range(20):
                ps_w = psA.tile([128, 256], F32, space="PSUM",
                                tag="ps_a" if wi % 2 == 0 else "ps_b",
                                name="ps_w", bufs=3)
                nc.tensor.matmul(out=ps_w[:1, :], lhsT=wu[:, :1],
                                 rhs=wu[:, :256], start=True, stop=True)

            # biases + index grids (resident; off the sync queue, which is
            # reserved for the bulk x/w stream)
            b1ot = sbuf.tile([128, 2 * MF], F32, tag="b1ot", name="b1ot",
                             bufs=1)
            b1bt = sbuf.tile([128, 2 * MF], F32, tag="b1bt", name="b1bt",
                             bufs=1)
            sb1t = sbuf.tile([128, 2 * MF], F32, tag="sb1t", name="sb1t",
                             bufs=1)
            cwt = sbuf.tile([128, G], F32, tag="cwt", name="cwt", bufs=1)
            sct = sbuf.tile([128, G], I32, tag="sct", name="sct", bufs=1)
            b2ot = sbuf.tile([128, D], F32, tag="b2ot", name="b2ot", bufs=1)
            b2bt = sbuf.tile([128, D], F32, tag="b2bt", name="b2bt", bufs=1)
            sb2t = sbuf.tile([128, D], F32, tag="sb2t", name="sb2t", bufs=1)
            nc.scalar.dma_start(out=b1ot[:], in_=b1o[:])
            nc.gpsimd.dma_start(out=b1bt[:], in_=b1b[:])
            nc.gpsimd.dma_start(out=cwt[:], in_=cwd[:])
            nc.gpsimd.dma_start(out=sct[:], in_=scat[:])
            nc.gpsimd.dma_start(out=sb1t[:], in_=sb1[:])

            def mk_fin(cbase, y_tiles, a2a_in, yh_box):
                def fin_tile(q, t, rows, d, ds, last):
                    # *combine weight (scalar engine), scatter to a2a buf
                    if q != N_SLAB - 1:
                        return
                    col = cbase + t
                    if d == 0:
                        yh_box[0] = sbuf.tile([128, D], F16, tag="yh",
                                              name="yh", bufs=2)
                    yh = yh_box[0]
                    nc.scalar.activation(yh[:rows, ds], y_tiles[t][:rows, ds],
                                         mybir.ActivationFunctionType.Copy,
                                         scale=cwt[:rows, col:col + 1])
                    if last:
                        nc.gpsimd.indirect_dma_start(
                            out=a2a_in[:],
                            out_offset=bass.IndirectOffsetOnAxis(
                                ap=sct[:rows, col:col + 1], axis=0),
                            in_=yh[:rows, :],
                            in_offset=None,
                            bounds_check=rows_h - 1,
                            oob_is_err=False,
                        )
                return fin_tile

            # ---------------- routed expert (2 half-passes over tokens) ----
            x_eng3 = [(nc.sync, nc.scalar), (nc.gpsimd, nc.sync),
                      (nc.scalar, nc.gpsimd)]

            # Borrowed-stream head start: the scheduler gates each w1 tile
            # DMA on PE progress (slot reuse), so queue lookahead is capped
            # at `bufs` and evaporates at the half-0/half-1 boundary right
            # when the first AllToAll saturates DMA. The borrowed tags are
            # virgin during half 0, so their first tiles + x prefetch
            # ungated, emitted mid-half-0 (clear of the startup window).
            w1b_pre = {}
            xgb_pre = []

            def h0_mid():
                for m in range(5):
                    w1t = sbuf.tile([128, 2, KD, 128], F16, tag="w1t1",
                                    name="w1t1", bufs=5)
                    nc.gpsimd.dma_start(out=w1t[:], in_=w1b[m])
                    w1b_pre[m] = w1t
                KH = KD // 2
                for ci in range(n_chB):
                    xa = sbuf.tile([128, KH, 512], F16, tag=f"x1c{ci}a",
                                   name=f"x1c{ci}a", bufs=1)
                    xb = sbuf.tile([128, KH, 512], F16, tag=f"x1c{ci}b",
                                   name=f"x1c{ci}b", bufs=1)
                    nc.gpsimd.dma_start(out=xa[:], in_=xgb[ci][:, 0:KH])
                    nc.gpsimd.dma_start(out=xb[:], in_=xgb[ci][:, KH:KD])
                    xgb_pre.append((xa, xb))

            for h in range(2):
                y_tiles = [sbuf.tile([128, D], F16, tag=f"ya{t}",
                                     name=f"ya{t}", bufs=1)
                           for t in range(n_thR)]
                own = _Group([xg0, xg1][h], hR, w1o, w2o, b1ot, b2ot,
                             y_tiles,
                             mk_fin(h * n_thR, y_tiles, a2a_ins[h], [None]),
                             nc.sync, (nc.sync, nc.scalar))
                grps = [own]
                if h == 1:
                    yb_tiles = [sbuf.tile([128, D], F16, tag=f"yb{t}",
                                          name=f"yb{t}", bufs=1)
                                for t in range(n_thB)]
                    grps.append(_Group(
                        xgb, c_b, w1b, w2b, b1bt, b2bt, yb_tiles,
                        mk_fin(2 * n_thR, yb_tiles, a2a_ins[1], [None]),
                        nc.sync, (nc.gpsimd, nc.gpsimd),
                        w1_pre=w1b_pre, x_pre=xgb_pre))
                pre = ((lambda: nc.gpsimd.dma_start(out=b2ot[:], in_=b2o[:]))
                       if h == 0 else
                       (lambda: nc.gpsimd.dma_start(out=b2bt[:], in_=b2b[:])))
                _ffn_phase(nc, sbuf, psA, psB, grps, x_eng3, pre_loads=pre,
                           mid_loads=h0_mid if h == 0 else None)
                # dispatch this half back to the combiner cores
                nc.gpsimd.collective_compute(
                    "AllToAll",
                    mybir.AluOpType.bypass,
                    replica_groups=[list(range(NCORES))],
                    ins=[a2a_ins[h][:].opt()],
                    outs=[a2a_out[h * rows_h:(h + 1) * rows_h, :].opt()],
                )

            # ---------------- shared expert on owned tokens (overlaps) ----
            i0 = sbuf.tile([128, n_to], I32, tag="i0", name="i0", bufs=1)
            i1 = sbuf.tile([128, n_to], I32, tag="i1", name="i1", bufs=1)
            nc.scalar.dma_start(out=i0[:], in_=g0i[:])
            nc.scalar.dma_start(out=i1[:], in_=g1i[:])
            # gathers of routed contributions are emitted via pre_loads so
            # the gpsimd queue serves the shared phase's x pieces first;
            # they overlap shared compute (waiting on both AllToAlls via
            # the a2a_out dependency)
            r0s, r1s = [], []

            def shared_pre():
                nc.gpsimd.dma_start(out=sb2t[:], in_=sb2[:])
                for t in range(n_to):
                    r0 = sbuf.tile([128, D], F16, tag="r0", name="r0", bufs=4)
                    r1 = sbuf.tile([128, D], F16, tag="r1", name="r1", bufs=4)
                    nc.gpsimd.indirect_dma_start(
                        out=r0[:], out_offset=None, in_=a2a_out[:],
                        in_offset=bass.IndirectOffsetOnAxis(
                            ap=i0[:, t:t + 1], axis=0))
                    nc.gpsimd.indirect_dma_start(
                        out=r1[:], out_offset=None, in_=a2a_out[:],
                        in_offset=bass.IndirectOffsetOnAxis(
                            ap=i1[:, t:t + 1], axis=0))
                    r0s.append(r0)
                    r1s.append(r1)

            ys_tiles = [sbuf.tile([128, D], F16, tag=f"ya{t}", name=f"ya{t}",
                                  bufs=1)
                        for t in range(n_to)]

            def comb_tile(q, t, rows, d, ds, last):
                # routed contributions fold in one slab early (commutative);
                # after the final slab only a half-tile store trails the
                # last matmul
                if q == N_SLAB - 2:
                    yt = ys_tiles[t]
                    nc.vector.tensor_add(yt[:, ds], yt[:, ds], r0s[t][:, ds])
                    nc.vector.tensor_add(yt[:, ds], yt[:, ds], r1s[t][:, ds])
                elif q == N_SLAB - 1:
                    tr = slice(t * 128, (t + 1) * 128)
                    if t == n_to - 1 and last:
                        h0 = slice(ds.start, ds.start + 256)
                        h1 = slice(ds.start + 256, ds.stop)
                        nc.sync.dma_start(out=out[tr, h0],
                                          in_=ys_tiles[t][:, h0])
                        nc.scalar.dma_start(out=out[tr, h1],
                                            in_=ys_tiles[t][:, h1])
                    else:
                        eng = nc.sync if (t + d) % 2 == 0 else nc.scalar
                        eng.dma_start(out=out[tr, ds],
                                      in_=ys_tiles[t][:, ds])

            shared_g = _Group(xs, TO, sw1, sw2, sb1t, sb2t, ys_tiles,
                              comb_tile, nc.sync, (nc.sync, nc.scalar))
            _ffn_phase(nc, sbuf, psA, psB, [shared_g], x_eng3,
                       pre_loads=shared_pre)

    nc.compile()
    _nc_cache[key] = nc
    return nc


def _route(x, gate_w, gate_b):
    """Host gate: softmax top-2 (float64 for stable ordering)."""
    logits = (x.astype(np.float64) @ gate_w.astype(np.float64)
              + gate_b.astype(np.float64))
    m = logits.max(axis=-1, keepdims=True)
    p = np.exp(logits - m)
    p /= p.sum(axis=-1, keepdims=True)
    order = np.argsort(-p, axis=-1)
    idx = order[:, :TOPK]                      # [T, 2]
    wts = np.take_along_axis(p, idx, axis=-1)  # [T, 2]
    return idx, wts.astype(np.float32)


def _solve_caps(loads):
    """min R+B s.t. sum_e ceil((c_e-R)+ / B) <= NCORES, caps mult of 4."""
    best = None
    for R in range(1024, max(loads) + 4, 4):
        ovf = [c - R for c in loads if c > R]
        if not ovf:
            if best is None or R + 8 < best[0] + best[1]:
                best = (R, 8)
            break
        B = 4
        while B <= R:
            if sum(-(-o // B) for o in ovf) <= NCORES:
                break
            B += 4
        else:
            continue
        if best is None or R + B < best[0] + best[1]:
            best = (R, B)
    return best


def _tile_w1(w):      # [D, 2F] -> [MF, 128, 2, KD, 128]
    v = np.asarray(w, np.float16).reshape(KD, 128, 2 * MF, 128)
    s = np.stack([v[:, :, :MF, :], v[:, :, MF:, :]], axis=0)  # [j,k,p,m,c]
    return np.ascontiguousarray(s.transpose(3, 2, 0, 1, 4))


def _tile_w2(w):      # [F, D] -> [N_SLAB, 128, PER_SLAB, D]
    v = np.asarray(w, np.float16).reshape(N_SLAB, PER_SLAB, 128, D)
    return np.ascontiguousarray(v.transpose(0, 2, 1, 3))


def _col_bias(b):     # [2F] -> [128, 2*MF]
    return np.ascontiguousarray(
        np.asarray(b, np.float32).reshape(2 * MF, 128).T)


def _xT_blocks(xr, c_cap):
    """[C, D] fp16 -> per-chunk contiguous blocks [n_ch, 128, KD, 512]."""
    ct = np.zeros((128, KD, c_cap), np.float16)
    n = len(xr)
    if n:
        ct[:, :, :n] = xr.T.reshape(KD, 128, n).transpose(1, 0, 2)
    chunks = _chunk_slices(c_cap)
    blk = np.zeros((len(chunks), 128, KD, 512), np.float16)
    for ci, (cs, cw) in enumerate(chunks):
        blk[ci, :, :, :cw] = ct[:, :, cs:cs + cw]
    return blk


def kernel(x, gate_w, gate_b, shared_w1, shared_b1, shared_w2, shared_b2,
           routed_w1, routed_b1, routed_w2, routed_b2):
    x = np.asarray(x, dtype=np.float32)
    topk_idx, topk_w = _route(x, np.asarray(gate_w), np.asarray(gate_b))

    owner = np.arange(T) // TO                 # owning core per token

    # per-expert dispatch lists, ordered by (owner, token)
    tok_lists, wt_lists = [], []
    for e in range(E):
        sel = (topk_idx == e)                  # [T, 2]
        tsel = np.nonzero(sel.any(axis=1))[0]  # ascending => owner-sorted
        k_of = sel[tsel, 1].astype(np.int64)   # slot (experts distinct)
        w_of = topk_w[tsel, :][np.arange(len(tsel)), k_of]
        tok_lists.append(tsel)
        wt_lists.append(w_of)

    loads = [len(t) for t in tok_lists]
    c_r, c_b = _solve_caps(loads)
    hR = c_r // 2
    n_thR = _n_tiles(hR)
    n_thB = _n_tiles(c_b)
    G = 2 * n_thR + n_thB

    # borrowed pieces: overflow of hot experts, one piece per core
    empty_t = np.zeros(0, np.int64)
    empty_w = np.zeros(0, np.float32)
    pieces = []
    for e in range(E):
        for s in range(min(loads[e], c_r), loads[e], c_b):
            pieces.append((e, tok_lists[e][s:s + c_b],
                           wt_lists[e][s:s + c_b]))
    assert len(pieces) <= NCORES, (loads, c_r, c_b)
    # per-core groups: [half0 own, half1 own, half1 borrowed]
    core_groups = []
    for c in range(NCORES):
        bw = pieces[c] if c < len(pieces) else (c, empty_t, empty_w)
        own_t, own_w = tok_lists[c], wt_lists[c]
        core_groups.append([(c, own_t[:hR], own_w[:hR]),
                            (c, own_t[hR:c_r], own_w[hR:c_r]), bw])

    # a2a row cap: per (src core, half, dest core) group sizes; half 1
    # packs own + borrowed tokens jointly per dest
    p0 = 1
    for c in range(NCORES):
        g0, g1, gb = core_groups[c]
        cnt0 = np.bincount(owner[g0[1]], minlength=NCORES)
        cnt1 = (np.bincount(owner[g1[1]], minlength=NCORES)
                + np.bincount(owner[gb[1]], minlength=NCORES))
        p0 = max(p0, int(cnt0.max()), int(cnt1.max()))
    p0 = -(-p0 // 8) * 8
    rows_h = NCORES * p0

    nc = _build(c_r, c_b, p0)

    # host-side layouts (fp16 compute dtype)
    w1r = np.asarray(routed_w1, np.float16)              # [E, D, 2F]
    w2r = np.asarray(routed_w2, np.float16)              # [E, F, D]
    b1r = np.asarray(routed_b1)                          # [E, 2F]
    b2r = np.asarray(routed_b2, np.float32)              # [E, D]
    xr = x.astype(np.float16)                            # [T, D]

    sw1_t = _tile_w1(np.asarray(shared_w1, np.float16)[0])
    sw2_t = _tile_w2(np.asarray(shared_w2, np.float16)[0])
    sb1_t = _col_bias(np.asarray(shared_b1)[0])

    # absolute a2a_out row for each (token, slot); combine-wt/scatter grids.
    # Grid columns: half0 own tiles [0, n_thR), half1 own [n_thR, 2*n_thR),
    # half1 borrowed [2*n_thR, G). Scatter row (within the half's a2a_in)
    # = dest*p0 + pos, pos running jointly over (own, borrowed) per dest.
    slot_rows = np.zeros((T, TOPK), np.int64)
    grids_cw = [np.zeros((128, G), np.float32) for _ in range(NCORES)]
    grids_sc = [np.full((128, G), 2**31 - 1, np.int32) for _ in range(NCORES)]
    for c in range(NCORES):
        for h in range(2):
            if h == 0:
                parts = [(0, core_groups[c][0])]
            else:
                parts = [(n_thR, core_groups[c][1]),
                         (2 * n_thR, core_groups[c][2])]
            next_pos = np.zeros(NCORES, np.int64)
            for cbase, (e, toks, wts) in parts:
                if len(toks) == 0:
                    continue
                own = owner[toks]
                pos = np.zeros(len(toks), np.int64)
                for o in range(NCORES):
                    m = own == o
                    pos[m] = next_pos[o] + np.arange(m.sum())
                    next_pos[o] += m.sum()
                jr = np.arange(len(toks)) % 128
                jc = np.arange(len(toks)) // 128
                grids_sc[c][jr, cbase + jc] = own * p0 + pos
                grids_cw[c][jr, cbase + jc] = wts
                abs_rows = h * rows_h + c * p0 + pos
                sel = (topk_idx[toks] == e)
                k_of = sel[:, 1].astype(np.int64)
                slot_rows[toks, k_of] = abs_rows

    in_maps = []
    for c in range(NCORES):
        (_, h0_toks, _), (_, h1_toks, _), (be, b_toks, _) = core_groups[c]

        g0 = np.ascontiguousarray(
            slot_rows[c * TO:(c + 1) * TO, 0].astype(np.int32)
            .reshape(TO // 128, 128).T)
        g1 = np.ascontiguousarray(
            slot_rows[c * TO:(c + 1) * TO, 1].astype(np.int32)
            .reshape(TO // 128, 128).T)

        in_maps.append({
            "xg0": _xT_blocks(xr[h0_toks], hR),
            "xg1": _xT_blocks(xr[h1_toks], hR),
            "xgb": _xT_blocks(xr[b_toks], c_b),
            "xs": _xT_blocks(xr[c * TO:(c + 1) * TO], TO),
            "w1o": _tile_w1(w1r[c]), "w2o": _tile_w2(w2r[c]),
            "w1b": _tile_w1(w1r[be]), "w2b": _tile_w2(w2r[be]),
            "sw1": sw1_t, "sw2": sw2_t,
            "b1o": _col_bias(b1r[c]), "b1b": _col_bias(b1r[be]),
            "sb1": sb1_t,
            "b2o": np.ascontiguousarray(np.broadcast_to(b2r[c], (128, D))),
            "b2b": np.ascontiguousarray(np.broadcast_to(b2r[be], (128, D))),
            "sb2": np.ascontiguousarray(np.broadcast_to(
                np.asarray(shared_b2, np.float32)[0], (128, D))),
            "cw": grids_cw[c], "scat": grids_sc[c], "g0i": g0, "g1i": g1,
        })

    res = run_bass_kernel_spmd(nc, in_maps, list(range(NCORES)))
    return np.concatenate([res.results[c]["out"] for c in range(NCORES)],
                          axis=0).astype(np.float32)


# revision 27
# speedup vs baseline: 1.0478x; 1.0020x over previous
"""MoE (top-2 of 8 experts + 1 shared expert, SwiGLU FFN) on 8 TRN2 NeuronCores.

Strategy (expert-parallel + load-balanced overflow, per the sharding hint):
  - Host computes the (tiny) gate: softmax top-2 over E=8 for T=8192 tokens,
    and from it the dispatch. >99.9% of FLOPs (the FFNs) run on device.
  - Load balancing: expert loads vary (~1932..2182 for T=8192); SPMD padding
    to the max would cost ~5%. Instead each core runs its own expert's first
    R tokens (R ~ 1980, two half-passes) plus a small "borrowed" group
    (cap cB ~ 104) of overflow tokens from some hot expert, with that
    expert's weights supplied per-core. (R, cB) solve
    min R+cB s.t. sum_e ceil((c_e-R)+ / cB) <= 8, bringing per-core routed
    work from max_e c_e (~2184) down to ~mean+2% (~2084).
  - The borrowed group rides INSIDE half-pass 1 as a second token group:
    its weight stream (w1b/w2b, 24MB) amortizes over the whole half-pass.
    A standalone borrowed pass would need >500GB/s of weight bandwidth.
  - Each half-pass scales rows by the gate weight and scatters rows into an
    AllToAll dispatch buffer laid out by destination core; each half's
    AllToAll fires when ready, overlapping remaining compute. Each core also
    runs the shared expert on its own T/8 token slice.
  - Combine on device: out[t] = shared(t) + contrib0(t) + contrib1(t), where
    contribs are indirect-gathered from the a2a output segments (host
    precomputes absolute rows; placement is fully host-controlled).

Compute dtype is fp16 (PSUM accumulation fp32). The PE clock is power-capped
at ~1.95 GHz sustained (GPIO throttle k=13/16), so the kernel is tuned to
keep the MM stream dense: all DMAs are large partition-major transfers
(weights are host-tiled so each f-chunk / w2-slab is a single DMA), biases
are folded into the PSUM->SBUF drain, the per-row gate scale runs on the
scalar engine, and the finalize/combine work is interleaved per-tile with
the last stage-2 slab so nothing serializes after the final matmul. At
phase start, x chunks >=1 and the slab-0 w2 load are emitted behind the
first weight tiles so the first matmuls aren't queued behind bulk DMAs.
"""
import contextlib

import numpy as np

import concourse.bass as bass
import concourse.tile as tile
from concourse import bacc, mybir
from concourse.bass_utils import run_bass_kernel_spmd

# problem shape (hardcoded per contract)
T = 8192
D = 1024
F = 4096
E = 8
TOPK = 2
NCORES = 8
TO = T // NCORES          # tokens owned per core

F32 = mybir.dt.float32
F16 = mybir.dt.float16
I32 = mybir.dt.int32

MF = 2 * F // 128 // 2    # 32 f-chunks (a-half; b-half is mp+MF)
N_SLAB = 4
PER_SLAB = MF // N_SLAB   # 8 f-chunks per slab
KD = D // 128             # 8 contraction chunks for stage 1

_nc_cache: dict[tuple, object] = {}


def _chunk_slices(c_len):
    """Moving-dim chunks of <=512, each >=256 so LDWEIGHTS stays hidden.

    Largest chunk first: it gates the first matmul of a pass, and the
    extra matmul runway it provides covers the smaller chunks' transfers.
    """
    out = []
    rem = c_len
    while rem > 0:
        if rem <= 512:
            w = rem
        elif rem >= 768:
            w = 512
        else:
            w = rem - 256
        out.append(w)
        rem -= w
    out.sort(reverse=True)
    widths = []
    pos = 0
    for w in out:
        widths.append((pos, w))
        pos += w
    return widths


def _n_tiles(c_len):
    return -(-c_len // 128)


class _Group:
    """One token group within an FFN pass: its own x, weights, biases,
    output tiles and finalize callback."""

    def __init__(self, x_src, c_len, w1d, w2d, b1t, b2t, y_tiles, on_tile_d,
                 w1_eng, w2_engs, w1_pre=None, x_pre=None):
        self.x_src = x_src
        self.c_len = c_len
        self.w1d = w1d
        self.w2d = w2d
        self.b1t = b1t
        self.b2t = b2t
        self.y_tiles = y_tiles
        self.on_tile_d = on_tile_d
        self.w1_eng = w1_eng
        self.w2_engs = w2_engs
        self.w1_pre = w1_pre or {}   # {f-chunk: preloaded w1 tile}
        self.x_pre = x_pre           # preloaded [(xa, xb)] per chunk
        self.chunks = _chunk_slices(c_len)
        self.xk = []
        self.g_tiles = []
        self.w2t = None


def _ffn_phase(nc, sbuf, psA, psB, groups, x_eng, pre_loads=None,
               mid_loads=None):
    """Emit one SwiGLU FFN pass over one or more token groups.

    Per group: x_src is per-chunk DRAM APs [128, KD, 512]; w1d is DRAM
    [MF, 128, 2, KD, 128] fp16 (host-tiled; [...,0,...]=a, 1=b); w2d is
    DRAM [N_SLAB, 128, PER_SLAB, D] fp16; b1t SBUF [128, 2*MF] f32 (col m =
    bias for a-chunk m / b-chunk m-MF); b2t SBUF [128, D] f32 added once
    into y at slab 0; y_tiles receive FFN output + bias2;
    on_tile_d(q, t, rows, d, ds, last) runs after each stage-2 add.

    Every x chunk / k-half and each w2 half lives in its own tile so the
    DMAs carry no false whole-tile dependencies and transfer concurrently.
    Only the first chunk's x is loaded ahead of the first weight tile; the
    rest are deferred so the first matmul isn't queued behind bulk DMAs.
    """
    KH = KD // 2
    ei = 0
    deferred_x = []
    for gi, g in enumerate(groups):
        if g.x_pre is not None:
            g.xk = g.x_pre
            ei += len(g.chunks)
            continue
        g.xk = []
        for ci, (cs, cw) in enumerate(g.chunks):
            xa = sbuf.tile([128, KH, 512], F16, tag=f"x{gi}c{ci}a",
                           name=f"x{gi}c{ci}a", bufs=2 if gi == 0 else 1)
            xb = sbuf.tile([128, KH, 512], F16, tag=f"x{gi}c{ci}b",
                           name=f"x{gi}c{ci}b", bufs=2 if gi == 0 else 1)
            e0, e1 = x_eng[ei % len(x_eng)]
            ei += 1
            # the sync queue carries the weight stream: defer its x halves
            # so the first weight tiles aren't queued behind bulk x DMAs
            if gi == 0 and ci == 0:
                e0.dma_start(out=xa[:], in_=g.x_src[ci][:, 0:KH])
                e1.dma_start(out=xb[:], in_=g.x_src[ci][:, KH:KD])
            else:
                if e0 is not nc.sync:
                    e0.dma_start(out=xa[:], in_=g.x_src[ci][:, 0:KH])
                else:
                    deferred_x.append((e0, xa, g.x_src[ci], 0))
                if e1 is not nc.sync:
                    e1.dma_start(out=xb[:], in_=g.x_src[ci][:, KH:KD])
                else:
                    deferred_x.append((e1, xb, g.x_src[ci], 1))
            g.xk.append((xa, xb))
    if pre_loads is not None:
        pre_loads()

    for q in range(N_SLAB):
        for g in groups:
            g.g_tiles = []
        for fi in range(PER_SLAB):
            mp = q * PER_SLAB + fi
            if q == 1 and fi == 0 and mid_loads is not None:
                mid_loads()
            for gi, g in enumerate(groups):
                w1t = g.w1_pre.pop(mp, None)
                if w1t is None:
                    w1t = sbuf.tile([128, 2, KD, 128], F16, tag=f"w1t{gi}",
                                    name=f"w1t{gi}", bufs=8 if gi == 0 else 5)
                    g.w1_eng.dma_start(out=w1t[:], in_=g.w1d[mp])
                if q == 0 and fi == 0 and gi == 0:
                    # sync-side x halves of chunks >=1 ride behind the
                    # first weight tile in queue order (still ahead of
                    # their first consumers, emitted below)
                    for (e0, xt, src, hf) in deferred_x:
                        e0.dma_start(out=xt[:],
                                     in_=src[:, hf * KH:(hf + 1) * KH])
                g_t = sbuf.tile([128, g.c_len], F16, tag=f"g{gi}_{fi}",
                                name=f"g{gi}_{fi}", bufs=1)
                for ci, (cs, cw) in enumerate(g.chunks):
                    ps_a = psA.tile([128, 512], F32, space="PSUM", tag="ps_a",
                                    name="ps_a", bufs=3)
                    ps_b = psA.tile([128, 512], F32, space="PSUM", tag="ps_b",
                                    name="ps_b", bufs=3)
                    for k in range(KD):
                        rhs = g.xk[ci][k // KH][:, k % KH, :cw]
                        nc.tensor.matmul(out=ps_a[:, :cw],
                                         lhsT=w1t[:, 0, k, :], rhs=rhs,
                                         start=(k == 0), stop=(k == KD - 1))
                    for k in range(KD):
                        rhs = g.xk[ci][k // KH][:, k % KH, :cw]
                        nc.tensor.matmul(out=ps_b[:, :cw],
                                         lhsT=w1t[:, 1, k, :], rhs=rhs,
                                         start=(k == 0), stop=(k == KD - 1))
                    t_a = sbuf.tile([128, 512], F16, tag="t_a", name="t_a",
                                    bufs=6)
                    t_b = sbuf.tile([128, 512], F16, tag="t_b", name="t_b",
                                    bufs=6)
                    nc.scalar.activation(t_a[:, :cw], ps_a[:, :cw],
                                         mybir.ActivationFunctionType.Silu,
                                         bias=g.b1t[:, mp:mp + 1])
                    nc.scalar.activation(
                        t_b[:, :cw], ps_b[:, :cw],
                        mybir.ActivationFunctionType.Identity,
                        bias=g.b1t[:, mp + MF:mp + MF + 1])
                    nc.vector.tensor_mul(g_t[:, cs:cs + cw], t_a[:, :cw],
                                         t_b[:, :cw])
                g.g_tiles.append(g_t)
            # stage-2 weights for this slab (streamed during stage-1).
            # Slab 0's load is emitted mid-pass so it doesn't sit in front
            # of the first weight tiles in the queue at phase start.
            if fi == (5 if q == 0 else 0):
                for gi, g in enumerate(groups):
                    w2ta = sbuf.tile([128, PER_SLAB // 2, D], F16,
                                     tag=f"w2{gi}a", name=f"w2{gi}a", bufs=1)
                    w2tb = sbuf.tile([128, PER_SLAB // 2, D], F16,
                                     tag=f"w2{gi}b", name=f"w2{gi}b", bufs=1)
                    g.w2_engs[0].dma_start(out=w2ta[:],
                                           in_=g.w2d[q][:, 0:PER_SLAB // 2])
                    g.w2_engs[1].dma_start(
                        out=w2tb[:], in_=g.w2d[q][:, PER_SLAB // 2:PER_SLAB])
                    g.w2t = (w2ta, w2tb)
        # stage-2 partial: y (+)= g_slab.T @ w2_slab
        for g in groups:
            n_t = _n_tiles(g.c_len)
            w2ta, w2tb = g.w2t
            for t in range(n_t):
                rows = min(128, g.c_len - t * 128)
                ts = slice(t * 128, t * 128 + rows)
                for d in range(D // 512):
                    ds = slice(d * 512, (d + 1) * 512)
                    ps_y = psB.tile([128, 512], F32, space="PSUM",
                                    tag="ps_y", name="ps_y", bufs=2)
                    for fi in range(PER_SLAB):
                        w2s = w2ta if fi < PER_SLAB // 2 else w2tb
                        nc.tensor.matmul(
                            out=ps_y[:rows, :], lhsT=g.g_tiles[fi][:, ts],
                            rhs=w2s[:, fi % (PER_SLAB // 2), ds],
                            start=(fi == 0), stop=(fi == PER_SLAB - 1))
                    yt = g.y_tiles[t]
                    if q == 0:
                        nc.vector.tensor_add(yt[:rows, ds], ps_y[:rows, :],
                                             g.b2t[:rows, ds])
                    else:
                        nc.vector.tensor_add(yt[:rows, ds], yt[:rows, ds],
                                             ps_y[:rows, :])
                    g.on_tile_d(q, t, rows, d, ds, d == D // 512 - 1)


def _build(c_r, c_b, p0):
    """SPMD program: half-pass 0 (own expert, c_r/2 tokens), half-pass 1
    (own c_r/2 + borrowed c_b as a second group), shared pass. Each half
    scatters into its own AllToAll buffer; the shared pass gathers +
    combines + stores."""
    key = (c_r, c_b, p0)
    if key in _nc_cache:
        return _nc_cache[key]

    nc = bacc.Bacc("TRN2", target_bir_lowering=False, debug=False,
                   num_devices=NCORES)

    def din(name, shape, dt):
        return nc.dram_tensor(name, shape, dt, kind="ExternalInput").ap()

    hR = c_r // 2
    n_thR = _n_tiles(hR)                               # y tiles per R half
    n_thB = _n_tiles(c_b)
    n_to = TO // 128
    G = 2 * n_thR + n_thB                              # cw/scat grid columns
    rows_h = NCORES * p0                               # rows per half buffer

    n_chR = len(_chunk_slices(hR))
    n_chB = len(_chunk_slices(c_b))
    n_cs = len(_chunk_slices(TO))
    # gathered/owned tokens^T, one contiguous 512-wide block per chunk
    xg0 = din("xg0", [n_chR, 128, KD, 512], F16)
    xg1 = din("xg1", [n_chR, 128, KD, 512], F16)
    xgb = din("xgb", [n_chB, 128, KD, 512], F16)
    xs = din("xs", [n_cs, 128, KD, 512], F16)
    w1o = din("w1o", [MF, 128, 2, KD, 128], F16)
    w2o = din("w2o", [N_SLAB, 128, PER_SLAB, D], F16)
    w1b = din("w1b", [MF, 128, 2, KD, 128], F16)
    w2b = din("w2b", [N_SLAB, 128, PER_SLAB, D], F16)
    sw1 = din("sw1", [MF, 128, 2, KD, 128], F16)
    sw2 = din("sw2", [N_SLAB, 128, PER_SLAB, D], F16)
    b1o = din("b1o", [128, 2 * MF], F32)
    b1b = din("b1b", [128, 2 * MF], F32)
    sb1 = din("sb1", [128, 2 * MF], F32)
    b2o = din("b2o", [128, D], F32)
    b2b = din("b2b", [128, D], F32)
    sb2 = din("sb2", [128, D], F32)
    cwd = din("cw", [128, G], F32)                     # combine wt per col
    scat = din("scat", [128, G], I32)                  # row in half's a2a_in
    g0i = din("g0i", [128, n_to], I32)                 # abs row in a2a_out
    g1i = din("g1i", [128, n_to], I32)
    out = nc.dram_tensor("out", [TO, D], F16, kind="ExternalOutput").ap()

    with tile.TileContext(nc) as tc:
        with contextlib.ExitStack() as ctx:
            sbuf = ctx.enter_context(tc.tile_pool(name="sbuf", bufs=1))
            psA = ctx.enter_context(tc.tile_pool(name="psA", bufs=3,
                                                 space="PSUM"))
            psB = ctx.enter_context(tc.tile_pool(name="psB", bufs=2,
                                                 space="PSUM"))
            dpool = ctx.enter_context(tc.tile_pool(name="dram", bufs=1,
                                                   space="DRAM"))

            a2a_in0 = dpool.tile([rows_h, D], F16)
            a2a_in1 = dpool.tile([rows_h, D], F16)
            a2a_out = dpool.tile([2 * rows_h, D], F16)
            a2a_ins = [a2a_in0, a2a_in1]

            # PE warmup: trip the HAM activity window during the input
            # DMAs so the first real matmuls run at full clock. Fed from a
            # memset tile so no DMA gates it. Sized to end about when the
            # first x/w tiles land (~6us) -- overshoot delays real work.
            wu = sbuf.tile([128, 512], F16, tag="wu", name="wu", bufs=1)
            nc.vector.memset(wu[:], 1.0)
            for wi in range(24):
                ps_w = psA.tile([128, 256], F32, space="PSUM",
                                tag="ps_a" if wi % 2 == 0 else "ps_b",
                                name="ps_w", bufs=3)
                nc.tensor.matmul(out=ps_w[:1, :], lhsT=wu[:, :1],
                                 rhs=wu[:, :256], start=True, stop=True)

            # first own w1 tile rides the (otherwise idle) gpsimd queue so
            # the first real matmul needs three parallel 0.5MB transfers
            # (x0c0 a+b, w1 tile 0) instead of a serial sync-queue chain
            w1o0 = sbuf.tile([128, 2, KD, 128], F16, tag="w1t0",
                             name="w1t0", bufs=8)
            nc.gpsimd.dma_start(out=w1o0[:], in_=w1o[0])

            # biases + index grids (resident; off the sync queue, which is
            # reserved for the bulk x/w stream)
            b1ot = sbuf.tile([128, 2 * MF], F32, tag="b1ot", name="b1ot",
                             bufs=1)
            b1bt = sbuf.tile([128, 2 * MF], F32, tag="b1bt", name="b1bt",
                             bufs=1)
            sb1t = sbuf.tile([128, 2 * MF], F32, tag="sb1t", name="sb1t",
                             bufs=1)
            cwt = sbuf.tile([128, G], F32, tag="cwt", name="cwt", bufs=1)
            sct = sbuf.tile([128, G], I32, tag="sct", name="sct", bufs=1)
            b2ot = sbuf.tile([128, D], F32, tag="b2ot", name="b2ot", bufs=1)
            b2bt = sbuf.tile([128, D], F32, tag="b2bt", name="b2bt", bufs=1)
            sb2t = sbuf.tile([128, D], F32, tag="sb2t", name="sb2t", bufs=1)
            nc.scalar.dma_start(out=b1ot[:], in_=b1o[:])
            nc.gpsimd.dma_start(out=b1bt[:], in_=b1b[:])
            nc.gpsimd.dma_start(out=cwt[:], in_=cwd[:])
            nc.gpsimd.dma_start(out=sct[:], in_=scat[:])
            nc.gpsimd.dma_start(out=sb1t[:], in_=sb1[:])

            def mk_fin(cbase, y_tiles, a2a_in, yh_box):
                def fin_tile(q, t, rows, d, ds, last):
                    # *combine weight (scalar engine), scatter to a2a buf
                    if q != N_SLAB - 1:
                        return
                    col = cbase + t
                    if d == 0:
                        yh_box[0] = sbuf.tile([128, D], F16, tag="yh",
                                              name="yh", bufs=2)
                    yh = yh_box[0]
                    nc.scalar.activation(yh[:rows, ds], y_tiles[t][:rows, ds],
                                         mybir.ActivationFunctionType.Copy,
                                         scale=cwt[:rows, col:col + 1])
                    if last:
                        nc.gpsimd.indirect_dma_start(
                            out=a2a_in[:],
                            out_offset=bass.IndirectOffsetOnAxis(
                                ap=sct[:rows, col:col + 1], axis=0),
                            in_=yh[:rows, :],
                            in_offset=None,
                            bounds_check=rows_h - 1,
                            oob_is_err=False,
                        )
                return fin_tile

            # ---------------- routed expert (2 half-passes over tokens) ----
            x_eng3 = [(nc.sync, nc.scalar), (nc.gpsimd, nc.sync),
                      (nc.scalar, nc.gpsimd)]

            # Borrowed-stream head start: the scheduler gates each w1 tile
            # DMA on PE progress (slot reuse), so queue lookahead is capped
            # at `bufs` and evaporates at the half-0/half-1 boundary right
            # when the first AllToAll saturates DMA. The borrowed tags are
            # virgin during half 0, so their first tiles + x prefetch
            # ungated, emitted mid-half-0 (clear of the startup window).
            w1b_pre = {}
            xgb_pre = []

            def h0_mid():
                for m in range(5):
                    w1t = sbuf.tile([128, 2, KD, 128], F16, tag="w1t1",
                                    name="w1t1", bufs=5)
                    nc.gpsimd.dma_start(out=w1t[:], in_=w1b[m])
                    w1b_pre[m] = w1t
                KH = KD // 2
                for ci in range(n_chB):
                    xa = sbuf.tile([128, KH, 512], F16, tag=f"x1c{ci}a",
                                   name=f"x1c{ci}a", bufs=1)
                    xb = sbuf.tile([128, KH, 512], F16, tag=f"x1c{ci}b",
                                   name=f"x1c{ci}b", bufs=1)
                    nc.gpsimd.dma_start(out=xa[:], in_=xgb[ci][:, 0:KH])
                    nc.gpsimd.dma_start(out=xb[:], in_=xgb[ci][:, KH:KD])
                    xgb_pre.append((xa, xb))

            for h in range(2):
                y_tiles = [sbuf.tile([128, D], F16, tag=f"ya{t}",
                                     name=f"ya{t}", bufs=1)
                           for t in range(n_thR)]
                own = _Group([xg0, xg1][h], hR, w1o, w2o, b1ot, b2ot,
                             y_tiles,
                             mk_fin(h * n_thR, y_tiles, a2a_ins[h], [None]),
                             nc.sync, (nc.sync, nc.scalar),
                             w1_pre={0: w1o0} if h == 0 else None)
                grps = [own]
                if h == 1:
                    yb_tiles = [sbuf.tile([128, D], F16, tag=f"yb{t}",
                                          name=f"yb{t}", bufs=1)
                                for t in range(n_thB)]
                    grps.append(_Group(
                        xgb, c_b, w1b, w2b, b1bt, b2bt, yb_tiles,
                        mk_fin(2 * n_thR, yb_tiles, a2a_ins[1], [None]),
                        nc.sync, (nc.gpsimd, nc.gpsimd),
                        w1_pre=w1b_pre, x_pre=xgb_pre))
                pre = ((lambda: nc.gpsimd.dma_start(out=b2ot[:], in_=b2o[:]))
                       if h == 0 else
                       (lambda: nc.gpsimd.dma_start(out=b2bt[:], in_=b2b[:])))
                _ffn_phase(nc, sbuf, psA, psB, grps, x_eng3, pre_loads=pre,
                           mid_loads=h0_mid if h == 0 else None)
                # dispatch this half back to the combiner cores
                nc.gpsimd.collective_compute(
                    "AllToAll",
                    mybir.AluOpType.bypass,
                    replica_groups=[list(range(NCORES))],
                    ins=[a2a_ins[h][:].opt()],
                    outs=[a2a_out[h * rows_h:(h + 1) * rows_h, :].opt()],
                )

            # ---------------- shared expert on owned tokens (overlaps) ----
            i0 = sbuf.tile([128, n_to], I32, tag="i0", name="i0", bufs=1)
            i1 = sbuf.tile([128, n_to], I32, tag="i1", name="i1", bufs=1)
            nc.scalar.dma_start(out=i0[:], in_=g0i[:])
            nc.scalar.dma_start(out=i1[:], in_=g1i[:])
            # gathers of routed contributions are emitted via pre_loads so
            # the gpsimd queue serves the shared phase's x pieces first;
            # they overlap shared compute (waiting on both AllToAlls via
            # the a2a_out dependency)
            r0s, r1s = [], []

            def shared_pre():
                nc.gpsimd.dma_start(out=sb2t[:], in_=sb2[:])
                for t in range(n_to):
                    r0 = sbuf.tile([128, D], F16, tag="r0", name="r0", bufs=4)
                    r1 = sbuf.tile([128, D], F16, tag="r1", name="r1", bufs=4)
                    nc.gpsimd.indirect_dma_start(
                        out=r0[:], out_offset=None, in_=a2a_out[:],
                        in_offset=bass.IndirectOffsetOnAxis(
                            ap=i0[:, t:t + 1], axis=0))
                    nc.gpsimd.indirect_dma_start(
                        out=r1[:], out_offset=None, in_=a2a_out[:],
                        in_offset=bass.IndirectOffsetOnAxis(
                            ap=i1[:, t:t + 1], axis=0))
                    r0s.append(r0)
                    r1s.append(r1)

            ys_tiles = [sbuf.tile([128, D], F16, tag=f"ya{t}", name=f"ya{t}",
                                  bufs=1)
                        for t in range(n_to)]

            def comb_tile(q, t, rows, d, ds, last):
                # routed contributions fold in one slab early (commutative);
                # after the final slab only a half-tile store trails the
                # last matmul
                if q == N_SLAB - 2:
                    yt = ys_tiles[t]
                    nc.vector.tensor_add(yt[:, ds], yt[:, ds], r0s[t][:, ds])
                    nc.vector.tensor_add(yt[:, ds], yt[:, ds], r1s[t][:, ds])
                elif q == N_SLAB - 1:
                    tr = slice(t * 128, (t + 1) * 128)
                    if t == n_to - 1 and last:
                        h0 = slice(ds.start, ds.start + 256)
                        h1 = slice(ds.start + 256, ds.stop)
                        nc.sync.dma_start(out=out[tr, h0],
                                          in_=ys_tiles[t][:, h0])
                        nc.scalar.dma_start(out=out[tr, h1],
                                            in_=ys_tiles[t][:, h1])
                    else:
                        eng = nc.sync if (t + d) % 2 == 0 else nc.scalar
                        eng.dma_start(out=out[tr, ds],
                                      in_=ys_tiles[t][:, ds])

            shared_g = _Group(xs, TO, sw1, sw2, sb1t, sb2t, ys_tiles,
                              comb_tile, nc.sync, (nc.sync, nc.scalar))
            _ffn_phase(nc, sbuf, psA, psB, [shared_g], x_eng3,
                       pre_loads=shared_pre)

    nc.compile()
    _nc_cache[key] = nc
    return nc


def _route(x, gate_w, gate_b):
    """Host gate: softmax top-2 (float64 for stable ordering)."""
    logits = (x.astype(np.float64) @ gate_w.astype(np.float64)
              + gate_b.astype(np.float64))
    m = logits.max(axis=-1, keepdims=True)
    p = np.exp(logits - m)
    p /= p.sum(axis=-1, keepdims=True)
    order = np.argsort(-p, axis=-1)
    idx = order[:, :TOPK]                      # [T, 2]
    wts = np.take_along_axis(p, idx, axis=-1)  # [T, 2]
    return idx, wts.astype(np.float32)


def _solve_caps(loads):
    """min R+B s.t. sum_e ceil((c_e-R)+ / B) <= NCORES, caps mult of 4."""
    best = None
    for R in range(1024, max(loads) + 4, 4):
        ovf = [c - R for c in loads if c > R]
        if not ovf:
            if best is None or R + 8 < best[0] + best[1]:
                best = (R, 8)
            break
        B = 4
        while B <= R:
            if sum(-(-o // B) for o in ovf) <= NCORES:
                break
            B += 4
        else:
            continue
        if best is None or R + B < best[0] + best[1]:
            best = (R, B)
    return best


def _tile_w1(w):      # [D, 2F] -> [MF, 128, 2, KD, 128]
    v = np.asarray(w, np.float16).reshape(KD, 128, 2 * MF, 128)
    s = np.stack([v[:, :, :MF, :], v[:, :, MF:, :]], axis=0)  # [j,k,p,m,c]
    return np.ascontiguousarray(s.transpose(3, 2, 0, 1, 4))


def _tile_w2(w):      # [F, D] -> [N_SLAB, 128, PER_SLAB, D]
    v = np.asarray(w, np.float16).reshape(N_SLAB, PER_SLAB, 128, D)
    return np.ascontiguousarray(v.transpose(0, 2, 1, 3))


def _col_bias(b):     # [2F] -> [128, 2*MF]
    return np.ascontiguousarray(
        np.asarray(b, np.float32).reshape(2 * MF, 128).T)


def _xT_blocks(xr, c_cap):
    """[C, D] fp16 -> per-chunk contiguous blocks [n_ch, 128, KD, 512]."""
    ct = np.zeros((128, KD, c_cap), np.float16)
    n = len(xr)
    if n:
        ct[:, :, :n] = xr.T.reshape(KD, 128, n).transpose(1, 0, 2)
    chunks = _chunk_slices(c_cap)
    blk = np.zeros((len(chunks), 128, KD, 512), np.float16)
    for ci, (cs, cw) in enumerate(chunks):
        blk[ci, :, :, :cw] = ct[:, :, cs:cs + cw]
    return blk


def kernel(x, gate_w, gate_b, shared_w1, shared_b1, shared_w2, shared_b2,
           routed_w1, routed_b1, routed_w2, routed_b2):
    x = np.asarray(x, dtype=np.float32)
    topk_idx, topk_w = _route(x, np.asarray(gate_w), np.asarray(gate_b))

    owner = np.arange(T) // TO                 # owning core per token

    # per-expert dispatch lists, ordered by (owner, token)
    tok_lists, wt_lists = [], []
    for e in range(E):
        sel = (topk_idx == e)                  # [T, 2]
        tsel = np.nonzero(sel.any(axis=1))[0]  # ascending => owner-sorted
        k_of = sel[tsel, 1].astype(np.int64)   # slot (experts distinct)
        w_of = topk_w[tsel, :][np.arange(len(tsel)), k_of]
        tok_lists.append(tsel)
        wt_lists.append(w_of)

    loads = [len(t) for t in tok_lists]
    c_r, c_b = _solve_caps(loads)
    hR = c_r // 2
    n_thR = _n_tiles(hR)
    n_thB = _n_tiles(c_b)
    G = 2 * n_thR + n_thB

    # borrowed pieces: overflow of hot experts, one piece per core
    empty_t = np.zeros(0, np.int64)
    empty_w = np.zeros(0, np.float32)
    pieces = []
    for e in range(E):
        for s in range(min(loads[e], c_r), loads[e], c_b):
            pieces.append((e, tok_lists[e][s:s + c_b],
                           wt_lists[e][s:s + c_b]))
    assert len(pieces) <= NCORES, (loads, c_r, c_b)
    # per-core groups: [half0 own, half1 own, half1 borrowed]
    core_groups = []
    for c in range(NCORES):
        bw = pieces[c] if c < len(pieces) else (c, empty_t, empty_w)
        own_t, own_w = tok_lists[c], wt_lists[c]
        core_groups.append([(c, own_t[:hR], own_w[:hR]),
                            (c, own_t[hR:c_r], own_w[hR:c_r]), bw])

    # a2a row cap: per (src core, half, dest core) group sizes; half 1
    # packs own + borrowed tokens jointly per dest
    p0 = 1
    for c in range(NCORES):
        g0, g1, gb = core_groups[c]
        cnt0 = np.bincount(owner[g0[1]], minlength=NCORES)
        cnt1 = (np.bincount(owner[g1[1]], minlength=NCORES)
                + np.bincount(owner[gb[1]], minlength=NCORES))
        p0 = max(p0, int(cnt0.max()), int(cnt1.max()))
    p0 = -(-p0 // 8) * 8
    rows_h = NCORES * p0

    nc = _build(c_r, c_b, p0)

    # host-side layouts (fp16 compute dtype)
    w1r = np.asarray(routed_w1, np.float16)              # [E, D, 2F]
    w2r = np.asarray(routed_w2, np.float16)              # [E, F, D]
    b1r = np.asarray(routed_b1)                          # [E, 2F]
    b2r = np.asarray(routed_b2, np.float32)              # [E, D]
    xr = x.astype(np.float16)                            # [T, D]

    sw1_t = _tile_w1(np.asarray(shared_w1, np.float16)[0])
    sw2_t = _tile_w2(np.asarray(shared_w2, np.float16)[0])
    sb1_t = _col_bias(np.asarray(shared_b1)[0])

    # absolute a2a_out row for each (token, slot); combine-wt/scatter grids.
    # Grid columns: half0 own tiles [0, n_thR), half1 own [n_thR, 2*n_thR),
    # half1 borrowed [2*n_thR, G). Scatter row (within the half's a2a_in)
    # = dest*p0 + pos, pos running jointly over (own, borrowed) per dest.
    slot_rows = np.zeros((T, TOPK), np.int64)
    grids_cw = [np.zeros((128, G), np.float32) for _ in range(NCORES)]
    grids_sc = [np.full((128, G), 2**31 - 1, np.int32) for _ in range(NCORES)]
    for c in range(NCORES):
        for h in range(2):
            if h == 0:
                parts = [(0, core_groups[c][0])]
            else:
                parts = [(n_thR, core_groups[c][1]),
                         (2 * n_thR, core_groups[c][2])]
            next_pos = np.zeros(NCORES, np.int64)
            for cbase, (e, toks, wts) in parts:
                if len(toks) == 0:
                    continue
                own = owner[toks]
                pos = np.zeros(len(toks), np.int64)
                for o in range(NCORES):
                    m = own == o
                    pos[m] = next_pos[o] + np.arange(m.sum())
                    next_pos[o] += m.sum()
                jr = np.arange(len(toks)) % 128
                jc = np.arange(len(toks)) // 128
                grids_sc[c][jr, cbase + jc] = own * p0 + pos
                grids_cw[c][jr, cbase + jc] = wts
                abs_rows = h * rows_h + c * p0 + pos
                sel = (topk_idx[toks] == e)
                k_of = sel[:, 1].astype(np.int64)
                slot_rows[toks, k_of] = abs_rows

    in_maps = []
    for c in range(NCORES):
        (_, h0_toks, _), (_, h1_toks, _), (be, b_toks, _) = core_groups[c]

        g0 = np.ascontiguousarray(
            slot_rows[c * TO:(c + 1) * TO, 0].astype(np.int32)
            .reshape(TO // 128, 128).T)
        g1 = np.ascontiguousarray(
            slot_rows[c * TO:(c + 1) * TO, 1].astype(np.int32)
            .reshape(TO // 128, 128).T)

        in_maps.append({
            "xg0": _xT_blocks(xr[h0_toks], hR),
            "xg1": _xT_blocks(xr[h1_toks], hR),
            "xgb": _xT_blocks(xr[b_toks], c_b),
            "xs": _xT_blocks(xr[c * TO:(c + 1) * TO], TO),
            "w1o": _tile_w1(w1r[c]), "w2o": _tile_w2(w2r[c]),
            "w1b": _tile_w1(w1r[be]), "w2b": _tile_w2(w2r[be]),
            "sw1": sw1_t, "sw2": sw2_t,
            "b1o": _col_bias(b1r[c]), "b1b": _col_bias(b1r[be]),
            "sb1": sb1_t,
            "b2o": np.ascontiguousarray(np.broadcast_to(b2r[c], (128, D))),
            "b2b": np.ascontiguousarray(np.broadcast_to(b2r[be], (128, D))),
            "sb2": np.ascontiguousarray(np.broadcast_to(
                np.asarray(shared_b2, np.float32)[0], (128, D))),
            "cw": grids_cw[c], "scat": grids_sc[c], "g0i": g0, "g1i": g1,
        })

    res = run_bass_kernel_spmd(nc, in_maps, list(range(NCORES)))
    return np.concatenate([res.results[c]["out"] for c in range(NCORES)],
                          axis=0).astype(np.float32)


# revision 32
# speedup vs baseline: 1.0486x; 1.0007x over previous
"""MoE (top-2 of 8 experts + 1 shared expert, SwiGLU FFN) on 8 TRN2 NeuronCores.

Strategy (expert-parallel + load-balanced overflow, per the sharding hint):
  - Host computes the (tiny) gate: softmax top-2 over E=8 for T=8192 tokens,
    and from it the dispatch. >99.9% of FLOPs (the FFNs) run on device.
  - Load balancing: expert loads vary (~1932..2182 for T=8192); SPMD padding
    to the max would cost ~5%. Instead each core runs its own expert's first
    R tokens (R ~ 1980, two half-passes) plus a small "borrowed" group
    (cap cB ~ 104) of overflow tokens from some hot expert, with that
    expert's weights supplied per-core. (R, cB) solve
    min R+cB s.t. sum_e ceil((c_e-R)+ / cB) <= 8, bringing per-core routed
    work from max_e c_e (~2184) down to ~mean+2% (~2084).
  - The borrowed group rides INSIDE half-pass 1 as a second token group:
    its weight stream (w1b/w2b, 24MB) amortizes over the whole half-pass.
    A standalone borrowed pass would need >500GB/s of weight bandwidth.
  - Each half-pass scales rows by the gate weight and scatters rows into an
    AllToAll dispatch buffer laid out by destination core; each half's
    AllToAll fires when ready, overlapping remaining compute. Each core also
    runs the shared expert on its own T/8 token slice.
  - Combine on device: out[t] = shared(t) + contrib0(t) + contrib1(t), where
    contribs are indirect-gathered from the a2a output segments (host
    precomputes absolute rows; placement is fully host-controlled).

Compute dtype is fp16 (PSUM accumulation fp32). The PE clock is power-capped
at ~1.95 GHz sustained (GPIO throttle k=13/16), so the kernel is tuned to
keep the MM stream dense: all DMAs are large partition-major transfers
(weights are host-tiled so each f-chunk / w2-slab is a single DMA), biases
are folded into the PSUM->SBUF drain, the per-row gate scale runs on the
scalar engine, and the finalize/combine work is interleaved per-tile with
the last stage-2 slab so nothing serializes after the final matmul. At
phase start, x chunks >=1 and the slab-0 w2 load are emitted behind the
first weight tiles so the first matmuls aren't queued behind bulk DMAs.
"""
import contextlib

import numpy as np

import concourse.bass as bass
import concourse.tile as tile
from concourse import bacc, mybir
from concourse.bass_utils import run_bass_kernel_spmd

# problem shape (hardcoded per contract)
T = 8192
D = 1024
F = 4096
E = 8
TOPK = 2
NCORES = 8
TO = T // NCORES          # tokens owned per core

F32 = mybir.dt.float32
F16 = mybir.dt.float16
I32 = mybir.dt.int32

MF = 2 * F // 128 // 2    # 32 f-chunks (a-half; b-half is mp+MF)
N_SLAB = 4
PER_SLAB = MF // N_SLAB   # 8 f-chunks per slab
KD = D // 128             # 8 contraction chunks for stage 1

_nc_cache: dict[tuple, object] = {}


def _chunk_slices(c_len):
    """Moving-dim chunks of <=512, each >=256 so LDWEIGHTS stays hidden.

    Largest chunk first: it gates the first matmul of a pass, and the
    extra matmul runway it provides covers the smaller chunks' transfers.
    """
    out = []
    rem = c_len
    while rem > 0:
        if rem <= 512:
            w = rem
        elif rem >= 768:
            w = 512
        else:
            w = rem - 256
        out.append(w)
        rem -= w
    out.sort(reverse=True)
    widths = []
    pos = 0
    for w in out:
        widths.append((pos, w))
        pos += w
    return widths


def _n_tiles(c_len):
    return -(-c_len // 128)


class _Group:
    """One token group within an FFN pass: its own x, weights, biases,
    output tiles and finalize callback."""

    def __init__(self, x_src, c_len, w1d, w2d, b1t, b2t, y_tiles, on_tile_d,
                 w1_eng, w2_engs, w1_pre=None, x_pre=None):
        self.x_src = x_src
        self.c_len = c_len
        self.w1d = w1d
        self.w2d = w2d
        self.b1t = b1t
        self.b2t = b2t
        self.y_tiles = y_tiles
        self.on_tile_d = on_tile_d
        self.w1_eng = w1_eng
        self.w2_engs = w2_engs
        self.w1_pre = w1_pre or {}   # {f-chunk: preloaded w1 tile}
        self.x_pre = x_pre           # preloaded [(xa, xb)] per chunk
        self.chunks = _chunk_slices(c_len)
        self.xk = []
        self.g_tiles = []
        self.w2t = None


def _ffn_phase(nc, sbuf, psA, psB, groups, x_eng, pre_loads=None,
               mid_loads=None, late_loads=None):
    """Emit one SwiGLU FFN pass over one or more token groups.

    Per group: x_src is per-chunk DRAM APs [128, KD, 512]; w1d is DRAM
    [MF, 128, 2, KD, 128] fp16 (host-tiled; [...,0,...]=a, 1=b); w2d is
    DRAM [N_SLAB, 128, PER_SLAB, D] fp16; b1t SBUF [128, 2*MF] f32 (col m =
    bias for a-chunk m / b-chunk m-MF); b2t SBUF [128, D] f32 added once
    into y at slab 0; y_tiles receive FFN output + bias2;
    on_tile_d(q, t, rows, d, ds, last) runs after each stage-2 add.

    Every x chunk / k-half and each w2 half lives in its own tile so the
    DMAs carry no false whole-tile dependencies and transfer concurrently.
    Only the first chunk's x is loaded ahead of the first weight tile; the
    rest are deferred so the first matmul isn't queued behind bulk DMAs.
    """
    KH = KD // 2
    ei = 0
    deferred_x = []
    for gi, g in enumerate(groups):
        if g.x_pre is not None:
            g.xk = g.x_pre
            ei += len(g.chunks)
            continue
        g.xk = []
        for ci, (cs, cw) in enumerate(g.chunks):
            xa = sbuf.tile([128, KH, 512], F16, tag=f"x{gi}c{ci}a",
                           name=f"x{gi}c{ci}a", bufs=2 if gi == 0 else 1)
            xb = sbuf.tile([128, KH, 512], F16, tag=f"x{gi}c{ci}b",
                           name=f"x{gi}c{ci}b", bufs=2 if gi == 0 else 1)
            e0, e1 = x_eng[ei % len(x_eng)]
            ei += 1
            # the sync queue carries the weight stream: defer its x halves
            # so the first weight tiles aren't queued behind bulk x DMAs
            if gi == 0 and ci == 0:
                e0.dma_start(out=xa[:], in_=g.x_src[ci][:, 0:KH])
                e1.dma_start(out=xb[:], in_=g.x_src[ci][:, KH:KD])
            else:
                if e0 is not nc.sync:
                    e0.dma_start(out=xa[:], in_=g.x_src[ci][:, 0:KH])
                else:
                    deferred_x.append((e0, xa, g.x_src[ci], 0))
                if e1 is not nc.sync:
                    e1.dma_start(out=xb[:], in_=g.x_src[ci][:, KH:KD])
                else:
                    deferred_x.append((e1, xb, g.x_src[ci], 1))
            g.xk.append((xa, xb))
    if pre_loads is not None:
        pre_loads()

    for q in range(N_SLAB):
        for g in groups:
            g.g_tiles = []
        for fi in range(PER_SLAB):
            mp = q * PER_SLAB + fi
            if q == 1 and fi == 0 and mid_loads is not None:
                mid_loads()
            for gi, g in enumerate(groups):
                w1t = g.w1_pre.pop(mp, None)
                if w1t is None:
                    w1t = sbuf.tile([128, 2, KD, 128], F16, tag=f"w1t{gi}",
                                    name=f"w1t{gi}", bufs=8 if gi == 0 else 5)
                    g.w1_eng.dma_start(out=w1t[:], in_=g.w1d[mp])
                if q == 0 and fi == 0 and gi == 0:
                    # sync-side x halves of chunks >=1 ride behind the
                    # first weight tile in queue order (still ahead of
                    # their first consumers, emitted below)
                    for (e0, xt, src, hf) in deferred_x:
                        e0.dma_start(out=xt[:],
                                     in_=src[:, hf * KH:(hf + 1) * KH])
                g_t = sbuf.tile([128, g.c_len], F16, tag=f"g{gi}_{fi}",
                                name=f"g{gi}_{fi}", bufs=1)
                for ci, (cs, cw) in enumerate(g.chunks):
                    ps_a = psA.tile([128, 512], F32, space="PSUM", tag="ps_a",
                                    name="ps_a", bufs=3)
                    ps_b = psA.tile([128, 512], F32, space="PSUM", tag="ps_b",
                                    name="ps_b", bufs=3)
                    for k in range(KD):
                        rhs = g.xk[ci][k // KH][:, k % KH, :cw]
                        nc.tensor.matmul(out=ps_a[:, :cw],
                                         lhsT=w1t[:, 0, k, :], rhs=rhs,
                                         start=(k == 0), stop=(k == KD - 1))
                    for k in range(KD):
                        rhs = g.xk[ci][k // KH][:, k % KH, :cw]
                        nc.tensor.matmul(out=ps_b[:, :cw],
                                         lhsT=w1t[:, 1, k, :], rhs=rhs,
                                         start=(k == 0), stop=(k == KD - 1))
                    t_a = sbuf.tile([128, 512], F16, tag="t_a", name="t_a",
                                    bufs=6)
                    t_b = sbuf.tile([128, 512], F16, tag="t_b", name="t_b",
                                    bufs=6)
                    nc.scalar.activation(t_a[:, :cw], ps_a[:, :cw],
                                         mybir.ActivationFunctionType.Silu,
                                         bias=g.b1t[:, mp:mp + 1])
                    nc.scalar.activation(
                        t_b[:, :cw], ps_b[:, :cw],
                        mybir.ActivationFunctionType.Identity,
                        bias=g.b1t[:, mp + MF:mp + MF + 1])
                    nc.vector.tensor_mul(g_t[:, cs:cs + cw], t_a[:, :cw],
                                         t_b[:, :cw])
                g.g_tiles.append(g_t)
            # stage-2 weights for this slab (streamed during stage-1).
            # Slab 0's load is emitted at the last moment (stage-2 needs it
            # only after fi==7) so it stays out of the startup DMA ramp.
            if fi == (7 if q == 0 else 0):
                if q == 0 and late_loads is not None:
                    late_loads()
                for gi, g in enumerate(groups):
                    w2ta = sbuf.tile([128, PER_SLAB // 2, D], F16,
                                     tag=f"w2{gi}a", name=f"w2{gi}a", bufs=1)
                    w2tb = sbuf.tile([128, PER_SLAB // 2, D], F16,
                                     tag=f"w2{gi}b", name=f"w2{gi}b", bufs=1)
                    g.w2_engs[0].dma_start(out=w2ta[:],
                                           in_=g.w2d[q][:, 0:PER_SLAB // 2])
                    g.w2_engs[1].dma_start(
                        out=w2tb[:], in_=g.w2d[q][:, PER_SLAB // 2:PER_SLAB])
                    g.w2t = (w2ta, w2tb)
        # stage-2 partial: y (+)= g_slab.T @ w2_slab
        for g in groups:
            n_t = _n_tiles(g.c_len)
            w2ta, w2tb = g.w2t
            for t in range(n_t):
                rows = min(128, g.c_len - t * 128)
                ts = slice(t * 128, t * 128 + rows)
                for d in range(D // 512):
                    ds = slice(d * 512, (d + 1) * 512)
                    ps_y = psB.tile([128, 512], F32, space="PSUM",
                                    tag="ps_y", name="ps_y", bufs=2)
                    for fi in range(PER_SLAB):
                        w2s = w2ta if fi < PER_SLAB // 2 else w2tb
                        nc.tensor.matmul(
                            out=ps_y[:rows, :], lhsT=g.g_tiles[fi][:, ts],
                            rhs=w2s[:, fi % (PER_SLAB // 2), ds],
                            start=(fi == 0), stop=(fi == PER_SLAB - 1))
                    yt = g.y_tiles[t]
                    if q == 0:
                        nc.vector.tensor_add(yt[:rows, ds], ps_y[:rows, :],
                                             g.b2t[:rows, ds])
                    else:
                        nc.vector.tensor_add(yt[:rows, ds], yt[:rows, ds],
                                             ps_y[:rows, :])
                    g.on_tile_d(q, t, rows, d, ds, d == D // 512 - 1)


def _build(c_r, c_b, p0):
    """SPMD program: half-pass 0 (own expert, c_r/2 tokens), half-pass 1
    (own c_r/2 + borrowed c_b as a second group), shared pass. Each half
    scatters into its own AllToAll buffer; the shared pass gathers +
    combines + stores."""
    key = (c_r, c_b, p0)
    if key in _nc_cache:
        return _nc_cache[key]

    nc = bacc.Bacc("TRN2", target_bir_lowering=False, debug=False,
                   num_devices=NCORES)

    def din(name, shape, dt):
        return nc.dram_tensor(name, shape, dt, kind="ExternalInput").ap()

    hR = c_r // 2
    n_thR = _n_tiles(hR)                               # y tiles per R half
    n_thB = _n_tiles(c_b)
    n_to = TO // 128
    G = 2 * n_thR + n_thB                              # cw/scat grid columns
    rows_h = NCORES * p0                               # rows per half buffer

    n_chR = len(_chunk_slices(hR))
    n_chB = len(_chunk_slices(c_b))
    n_cs = len(_chunk_slices(TO))
    # gathered/owned tokens^T, one contiguous 512-wide block per chunk
    xg0 = din("xg0", [n_chR, 128, KD, 512], F16)
    xg1 = din("xg1", [n_chR, 128, KD, 512], F16)
    xgb = din("xgb", [n_chB, 128, KD, 512], F16)
    xs = din("xs", [n_cs, 128, KD, 512], F16)
    w1o = din("w1o", [MF, 128, 2, KD, 128], F16)
    w2o = din("w2o", [N_SLAB, 128, PER_SLAB, D], F16)
    w1b = din("w1b", [MF, 128, 2, KD, 128], F16)
    w2b = din("w2b", [N_SLAB, 128, PER_SLAB, D], F16)
    sw1 = din("sw1", [MF, 128, 2, KD, 128], F16)
    sw2 = din("sw2", [N_SLAB, 128, PER_SLAB, D], F16)
    b1o = din("b1o", [128, 2 * MF], F32)
    b1b = din("b1b", [128, 2 * MF], F32)
    sb1 = din("sb1", [128, 2 * MF], F32)
    b2o = din("b2o", [128, D], F32)
    b2b = din("b2b", [128, D], F32)
    sb2 = din("sb2", [128, D], F32)
    cwd = din("cw", [128, G], F32)                     # combine wt per col
    scat = din("scat", [128, G], I32)                  # row in half's a2a_in
    g0i = din("g0i", [128, n_to], I32)                 # abs row in a2a_out
    g1i = din("g1i", [128, n_to], I32)
    out = nc.dram_tensor("out", [TO, D], F16, kind="ExternalOutput").ap()

    with tile.TileContext(nc) as tc:
        with contextlib.ExitStack() as ctx:
            sbuf = ctx.enter_context(tc.tile_pool(name="sbuf", bufs=1))
            psA = ctx.enter_context(tc.tile_pool(name="psA", bufs=3,
                                                 space="PSUM"))
            psB = ctx.enter_context(tc.tile_pool(name="psB", bufs=2,
                                                 space="PSUM"))
            dpool = ctx.enter_context(tc.tile_pool(name="dram", bufs=1,
                                                   space="DRAM"))

            a2a_in0 = dpool.tile([rows_h, D], F16)
            a2a_in1 = dpool.tile([rows_h, D], F16)
            a2a_out = dpool.tile([2 * rows_h, D], F16)
            a2a_ins = [a2a_in0, a2a_in1]

            # PE warmup: trip the HAM activity window during the input
            # DMAs so the first real matmuls run at full clock. Fed from a
            # memset tile so no DMA gates it. Sized to end about when the
            # first x/w tiles land (~6us) -- overshoot delays real work.
            wu = sbuf.tile([128, 512], F16, tag="wu", name="wu", bufs=1)
            nc.vector.memset(wu[:], 1.0)
            for wi in range(24):
                ps_w = psA.tile([128, 256], F32, space="PSUM",
                                tag="ps_a" if wi % 2 == 0 else "ps_b",
                                name="ps_w", bufs=3)
                nc.tensor.matmul(out=ps_w[:1, :], lhsT=wu[:, :1],
                                 rhs=wu[:, :256], start=True, stop=True)

            # first own w1 tile rides the (otherwise idle) gpsimd queue so
            # the first real matmul needs three parallel 0.5MB transfers
            # (x0c0 a+b, w1 tile 0) instead of a serial sync-queue chain
            w1o0 = sbuf.tile([128, 2, KD, 128], F16, tag="w1t0",
                             name="w1t0", bufs=8)
            nc.gpsimd.dma_start(out=w1o0[:], in_=w1o[0])

            # biases + index grids (resident; off the sync queue, which is
            # reserved for the bulk x/w stream)
            b1ot = sbuf.tile([128, 2 * MF], F32, tag="b1ot", name="b1ot",
                             bufs=1)
            b1bt = sbuf.tile([128, 2 * MF], F32, tag="b1bt", name="b1bt",
                             bufs=1)
            sb1t = sbuf.tile([128, 2 * MF], F32, tag="sb1t", name="sb1t",
                             bufs=1)
            cwt = sbuf.tile([128, G], F32, tag="cwt", name="cwt", bufs=1)
            sct = sbuf.tile([128, G], I32, tag="sct", name="sct", bufs=1)
            b2ot = sbuf.tile([128, D], F32, tag="b2ot", name="b2ot", bufs=1)
            b2bt = sbuf.tile([128, D], F32, tag="b2bt", name="b2bt", bufs=1)
            sb2t = sbuf.tile([128, D], F32, tag="sb2t", name="sb2t", bufs=1)
            # only b1o is needed in the first microseconds (half-0 drains);
            # the rest load mid-half-0, clear of the startup DMA ramp
            nc.scalar.dma_start(out=b1ot[:], in_=b1o[:])

            def mk_fin(cbase, y_tiles, a2a_in, yh_box):
                def fin_tile(q, t, rows, d, ds, last):
                    # *combine weight (scalar engine), scatter to a2a buf
                    if q != N_SLAB - 1:
                        return
                    col = cbase + t
                    if d == 0:
                        yh_box[0] = sbuf.tile([128, D], F16, tag="yh",
                                              name="yh", bufs=2)
                    yh = yh_box[0]
                    nc.scalar.activation(yh[:rows, ds], y_tiles[t][:rows, ds],
                                         mybir.ActivationFunctionType.Copy,
                                         scale=cwt[:rows, col:col + 1])
                    if last:
                        nc.gpsimd.indirect_dma_start(
                            out=a2a_in[:],
                            out_offset=bass.IndirectOffsetOnAxis(
                                ap=sct[:rows, col:col + 1], axis=0),
                            in_=yh[:rows, :],
                            in_offset=None,
                            bounds_check=rows_h - 1,
                            oob_is_err=False,
                        )
                return fin_tile

            # ---------------- routed expert (2 half-passes over tokens) ----
            x_eng3 = [(nc.sync, nc.scalar), (nc.gpsimd, nc.sync),
                      (nc.scalar, nc.gpsimd)]

            # Borrowed-stream head start: the scheduler gates each w1 tile
            # DMA on PE progress (slot reuse), so queue lookahead is capped
            # at `bufs` and evaporates at the half-0/half-1 boundary right
            # when the first AllToAll saturates DMA. The borrowed tags are
            # virgin during half 0, so their first tiles + x prefetch
            # ungated, emitted mid-half-0 (clear of the startup window).
            w1b_pre = {}
            xgb_pre = []

            def h0_mid():
                nc.gpsimd.dma_start(out=b1bt[:], in_=b1b[:])
                nc.gpsimd.dma_start(out=cwt[:], in_=cwd[:])
                nc.gpsimd.dma_start(out=sct[:], in_=scat[:])
                nc.gpsimd.dma_start(out=sb1t[:], in_=sb1[:])
                for m in range(5):
                    w1t = sbuf.tile([128, 2, KD, 128], F16, tag="w1t1",
                                    name="w1t1", bufs=5)
                    nc.gpsimd.dma_start(out=w1t[:], in_=w1b[m])
                    w1b_pre[m] = w1t
                KH = KD // 2
                for ci in range(n_chB):
                    xa = sbuf.tile([128, KH, 512], F16, tag=f"x1c{ci}a",
                                   name=f"x1c{ci}a", bufs=1)
                    xb = sbuf.tile([128, KH, 512], F16, tag=f"x1c{ci}b",
                                   name=f"x1c{ci}b", bufs=1)
                    nc.gpsimd.dma_start(out=xa[:], in_=xgb[ci][:, 0:KH])
                    nc.gpsimd.dma_start(out=xb[:], in_=xgb[ci][:, KH:KD])
                    xgb_pre.append((xa, xb))

            for h in range(2):
                y_tiles = [sbuf.tile([128, D], F16, tag=f"ya{t}",
                                     name=f"ya{t}", bufs=1)
                           for t in range(n_thR)]
                own = _Group([xg0, xg1][h], hR, w1o, w2o, b1ot, b2ot,
                             y_tiles,
                             mk_fin(h * n_thR, y_tiles, a2a_ins[h], [None]),
                             nc.sync, (nc.sync, nc.scalar),
                             w1_pre={0: w1o0} if h == 0 else None)
                grps = [own]
                if h == 1:
                    yb_tiles = [sbuf.tile([128, D], F16, tag=f"yb{t}",
                                          name=f"yb{t}", bufs=1)
                                for t in range(n_thB)]
                    grps.append(_Group(
                        xgb, c_b, w1b, w2b, b1bt, b2bt, yb_tiles,
                        mk_fin(2 * n_thR, yb_tiles, a2a_ins[1], [None]),
                        nc.sync, (nc.gpsimd, nc.gpsimd),
                        w1_pre=w1b_pre, x_pre=xgb_pre))
                late = ((lambda: nc.gpsimd.dma_start(out=b2ot[:], in_=b2o[:]))
                        if h == 0 else
                        (lambda: nc.gpsimd.dma_start(out=b2bt[:],
                                                     in_=b2b[:])))
                _ffn_phase(nc, sbuf, psA, psB, grps, x_eng3,
                           mid_loads=h0_mid if h == 0 else None,
                           late_loads=late)
                # dispatch this half back to the combiner cores
                nc.gpsimd.collective_compute(
                    "AllToAll",
                    mybir.AluOpType.bypass,
                    replica_groups=[list(range(NCORES))],
                    ins=[a2a_ins[h][:].opt()],
                    outs=[a2a_out[h * rows_h:(h + 1) * rows_h, :].opt()],
                )

            # ---------------- shared expert on owned tokens (overlaps) ----
            i0 = sbuf.tile([128, n_to], I32, tag="i0", name="i0", bufs=1)
            i1 = sbuf.tile([128, n_to], I32, tag="i1", name="i1", bufs=1)
            nc.scalar.dma_start(out=i0[:], in_=g0i[:])
            nc.scalar.dma_start(out=i1[:], in_=g1i[:])
            # gathers of routed contributions are emitted via pre_loads so
            # the gpsimd queue serves the shared phase's x pieces first;
            # they overlap shared compute (waiting on both AllToAlls via
            # the a2a_out dependency)
            r0s, r1s = [], []

            def shared_pre():
                nc.gpsimd.dma_start(out=sb2t[:], in_=sb2[:])
                for t in range(n_to):
                    r0 = sbuf.tile([128, D], F16, tag="r0", name="r0", bufs=4)
                    r1 = sbuf.tile([128, D], F16, tag="r1", name="r1", bufs=4)
                    nc.gpsimd.indirect_dma_start(
                        out=r0[:], out_offset=None, in_=a2a_out[:],
                        in_offset=bass.IndirectOffsetOnAxis(
                            ap=i0[:, t:t + 1], axis=0))
                    nc.gpsimd.indirect_dma_start(
                        out=r1[:], out_offset=None, in_=a2a_out[:],
                        in_offset=bass.IndirectOffsetOnAxis(
                            ap=i1[:, t:t + 1], axis=0))
                    r0s.append(r0)
                    r1s.append(r1)

            ys_tiles = [sbuf.tile([128, D], F16, tag=f"ya{t}", name=f"ya{t}",
                                  bufs=1)
                        for t in range(n_to)]

            def comb_tile(q, t, rows, d, ds, last):
                # routed contributions fold in one slab early (commutative);
                # after the final slab only a half-tile store trails the
                # last matmul
                if q == N_SLAB - 2:
                    yt = ys_tiles[t]
                    nc.vector.tensor_add(yt[:, ds], yt[:, ds], r0s[t][:, ds])
                    nc.vector.tensor_add(yt[:, ds], yt[:, ds], r1s[t][:, ds])
                elif q == N_SLAB - 1:
                    tr = slice(t * 128, (t + 1) * 128)
                    if t == n_to - 1 and last:
                        h0 = slice(ds.start, ds.start + 256)
                        h1 = slice(ds.start + 256, ds.stop)
                        nc.sync.dma_start(out=out[tr, h0],
                                          in_=ys_tiles[t][:, h0])
                        nc.scalar.dma_start(out=out[tr, h1],
                                            in_=ys_tiles[t][:, h1])
                    else:
                        eng = nc.sync if (t + d) % 2 == 0 else nc.scalar
                        eng.dma_start(out=out[tr, ds],
                                      in_=ys_tiles[t][:, ds])

            shared_g = _Group(xs, TO, sw1, sw2, sb1t, sb2t, ys_tiles,
                              comb_tile, nc.sync, (nc.sync, nc.scalar))
            _ffn_phase(nc, sbuf, psA, psB, [shared_g], x_eng3,
                       pre_loads=shared_pre)

    nc.compile()
    _nc_cache[key] = nc
    return nc


def _route(x, gate_w, gate_b):
    """Host gate: softmax top-2 (float64 for stable ordering)."""
    logits = (x.astype(np.float64) @ gate_w.astype(np.float64)
              + gate_b.astype(np.float64))
    m = logits.max(axis=-1, keepdims=True)
    p = np.exp(logits - m)
    p /= p.sum(axis=-1, keepdims=True)
    order = np.argsort(-p, axis=-1)
    idx = order[:, :TOPK]                      # [T, 2]
    wts = np.take_along_axis(p, idx, axis=-1)  # [T, 2]
    return idx, wts.astype(np.float32)


def _solve_caps(loads):
    """min R+B s.t. sum_e ceil((c_e-R)+ / B) <= NCORES, caps mult of 4."""
    best = None
    for R in range(1024, max(loads) + 4, 4):
        ovf = [c - R for c in loads if c > R]
        if not ovf:
            if best is None or R + 8 < best[0] + best[1]:
                best = (R, 8)
            break
        B = 4
        while B <= R:
            if sum(-(-o // B) for o in ovf) <= NCORES:
                break
            B += 4
        else:
            continue
        if best is None or R + B < best[0] + best[1]:
            best = (R, B)
    return best


def _tile_w1(w):      # [D, 2F] -> [MF, 128, 2, KD, 128]
    v = np.asarray(w, np.float16).reshape(KD, 128, 2 * MF, 128)
    s = np.stack([v[:, :, :MF, :], v[:, :, MF:, :]], axis=0)  # [j,k,p,m,c]
    return np.ascontiguousarray(s.transpose(3, 2, 0, 1, 4))


def _tile_w2(w):      # [F, D] -> [N_SLAB, 128, PER_SLAB, D]
    v = np.asarray(w, np.float16).reshape(N_SLAB, PER_SLAB, 128, D)
    return np.ascontiguousarray(v.transpose(0, 2, 1, 3))


def _col_bias(b):     # [2F] -> [128, 2*MF]
    return np.ascontiguousarray(
        np.asarray(b, np.float32).reshape(2 * MF, 128).T)


def _xT_blocks(xr, c_cap):
    """[C, D] fp16 -> per-chunk contiguous blocks [n_ch, 128, KD, 512]."""
    ct = np.zeros((128, KD, c_cap), np.float16)
    n = len(xr)
    if n:
        ct[:, :, :n] = xr.T.reshape(KD, 128, n).transpose(1, 0, 2)
    chunks = _chunk_slices(c_cap)
    blk = np.zeros((len(chunks), 128, KD, 512), np.float16)
    for ci, (cs, cw) in enumerate(chunks):
        blk[ci, :, :, :cw] = ct[:, :, cs:cs + cw]
    return blk


def kernel(x, gate_w, gate_b, shared_w1, shared_b1, shared_w2, shared_b2,
           routed_w1, routed_b1, routed_w2, routed_b2):
    x = np.asarray(x, dtype=np.float32)
    topk_idx, topk_w = _route(x, np.asarray(gate_w), np.asarray(gate_b))

    owner = np.arange(T) // TO                 # owning core per token

    # per-expert dispatch lists, ordered by (owner, token)
    tok_lists, wt_lists = [], []
    for e in range(E):
        sel = (topk_idx == e)                  # [T, 2]
        tsel = np.nonzero(sel.any(axis=1))[0]  # ascending => owner-sorted
        k_of = sel[tsel, 1].astype(np.int64)   # slot (experts distinct)
        w_of = topk_w[tsel, :][np.arange(len(tsel)), k_of]
        tok_lists.append(tsel)
        wt_lists.append(w_of)

    loads = [len(t) for t in tok_lists]
    c_r, c_b = _solve_caps(loads)
    hR = c_r // 2
    n_thR = _n_tiles(hR)
    n_thB = _n_tiles(c_b)
    G = 2 * n_thR + n_thB

    # borrowed pieces: overflow of hot experts, one piece per core
    empty_t = np.zeros(0, np.int64)
    empty_w = np.zeros(0, np.float32)
    pieces = []
    for e in range(E):
        for s in range(min(loads[e], c_r), loads[e], c_b):
            pieces.append((e, tok_lists[e][s:s + c_b],
                           wt_lists[e][s:s + c_b]))
    assert len(pieces) <= NCORES, (loads, c_r, c_b)
    # per-core groups: [half0 own, half1 own, half1 borrowed]
    core_groups = []
    for c in range(NCORES):
        bw = pieces[c] if c < len(pieces) else (c, empty_t, empty_w)
        own_t, own_w = tok_lists[c], wt_lists[c]
        core_groups.append([(c, own_t[:hR], own_w[:hR]),
                            (c, own_t[hR:c_r], own_w[hR:c_r]), bw])

    # a2a row cap: per (src core, half, dest core) group sizes; half 1
    # packs own + borrowed tokens jointly per dest
    p0 = 1
    for c in range(NCORES):
        g0, g1, gb = core_groups[c]
        cnt0 = np.bincount(owner[g0[1]], minlength=NCORES)
        cnt1 = (np.bincount(owner[g1[1]], minlength=NCORES)
                + np.bincount(owner[gb[1]], minlength=NCORES))
        p0 = max(p0, int(cnt0.max()), int(cnt1.max()))
    p0 = -(-p0 // 8) * 8
    rows_h = NCORES * p0

    nc = _build(c_r, c_b, p0)

    # host-side layouts (fp16 compute dtype)
    w1r = np.asarray(routed_w1, np.float16)              # [E, D, 2F]
    w2r = np.asarray(routed_w2, np.float16)              # [E, F, D]
    b1r = np.asarray(routed_b1)                          # [E, 2F]
    b2r = np.asarray(routed_b2, np.float32)              # [E, D]
    xr = x.astype(np.float16)                            # [T, D]

    sw1_t = _tile_w1(np.asarray(shared_w1, np.float16)[0])
    sw2_t = _tile_w2(np.asarray(shared_w2, np.float16)[0])
    sb1_t = _col_bias(np.asarray(shared_b1)[0])

    # absolute a2a_out row for each (token, slot); combine-wt/scatter grids.
    # Grid columns: half0 own tiles [0, n_thR), half1 own [n_thR, 2*n_thR),
    # half1 borrowed [2*n_thR, G). Scatter row (within the half's a2a_in)
    # = dest*p0 + pos, pos running jointly over (own, borrowed) per dest.
    slot_rows = np.zeros((T, TOPK), np.int64)
    grids_cw = [np.zeros((128, G), np.float32) for _ in range(NCORES)]
    grids_sc = [np.full((128, G), 2**31 - 1, np.int32) for _ in range(NCORES)]
    for c in range(NCORES):
        for h in range(2):
            if h == 0:
                parts = [(0, core_groups[c][0])]
            else:
                parts = [(n_thR, core_groups[c][1]),
                         (2 * n_thR, core_groups[c][2])]
            next_pos = np.zeros(NCORES, np.int64)
            for cbase, (e, toks, wts) in parts:
                if len(toks) == 0:
                    continue
                own = owner[toks]
                pos = np.zeros(len(toks), np.int64)
                for o in range(NCORES):
                    m = own == o
                    pos[m] = next_pos[o] + np.arange(m.sum())
                    next_pos[o] += m.sum()
                jr = np.arange(len(toks)) % 128
                jc = np.arange(len(toks)) // 128
                grids_sc[c][jr, cbase + jc] = own * p0 + pos
                grids_cw[c][jr, cbase + jc] = wts
                abs_rows = h * rows_h + c * p0 + pos
                sel = (topk_idx[toks] == e)
                k_of = sel[:, 1].astype(np.int64)
                slot_rows[toks, k_of] = abs_rows

    in_maps = []
    for c in range(NCORES):
        (_, h0_toks, _), (_, h1_toks, _), (be, b_toks, _) = core_groups[c]

        g0 = np.ascontiguousarray(
            slot_rows[c * TO:(c + 1) * TO, 0].astype(np.int32)
            .reshape(TO // 128, 128).T)
        g1 = np.ascontiguousarray(
            slot_rows[c * TO:(c + 1) * TO, 1].astype(np.int32)
            .reshape(TO // 128, 128).T)

        in_maps.append({
            "xg0": _xT_blocks(xr[h0_toks], hR),
            "xg1": _xT_blocks(xr[h1_toks], hR),
            "xgb": _xT_blocks(xr[b_toks], c_b),
            "xs": _xT_blocks(xr[c * TO:(c + 1) * TO], TO),
            "w1o": _tile_w1(w1r[c]), "w2o": _tile_w2(w2r[c]),
            "w1b": _tile_w1(w1r[be]), "w2b": _tile_w2(w2r[be]),
            "sw1": sw1_t, "sw2": sw2_t,
            "b1o": _col_bias(b1r[c]), "b1b": _col_bias(b1r[be]),
            "sb1": sb1_t,
            "b2o": np.ascontiguousarray(np.broadcast_to(b2r[c], (128, D))),
            "b2b": np.ascontiguousarray(np.broadcast_to(b2r[be], (128, D))),
            "sb2": np.ascontiguousarray(np.broadcast_to(
                np.asarray(shared_b2, np.float32)[0], (128, D))),
            "cw": grids_cw[c], "scat": grids_sc[c], "g0i": g0, "g1i": g1,
        })

    res = run_bass_kernel_spmd(nc, in_maps, list(range(NCORES)))
    return np.concatenate([res.results[c]["out"] for c in range(NCORES)],
                          axis=0).astype(np.float32)
